# revision 1
# baseline (speedup 1.0000x reference)
"""Trainium2 Bass kernel for a Neural Circuit Policies (LTC) cell.

Strategy: data-parallel over batch (32 batches -> 8 cores x 4). Per core the
T=32-step scan with 6 ODE unfolds runs as a fully unrolled serial chain.
Layout is dense-by-presynaptic-unit with PE "diagonal matmul" reductions:
  - z = sigma .* v_bc - sigma*mu   (DVE, broadcast APs; two pre-blocks:
    inter->cmd [128 x (4b,64j)], cmd->(cmd|motor) [64 x (4b,96j)])
  - sig = Sigmoid(z)               (ACT)
  - PE matmuls with static lhsT = [w*erev | w*mask] reduce over presynaptic
    partitions producing out[j', (b, j)]; the diagonal j'==j is the wanted
    weighted fan-in sum. A fused (psum + bias_row) * diag_mask then a strided
    reduce extract num/den per (post, batch). Leak/cm/eps constants ride in
    via the bias row.
  - v' = num * reciprocal(den)     (DVE)
Inter neurons receive no recurrent synapses -> affine per-unfold update with
per-step constants from the sensory tables. Sensory terms (parallel over T)
are precomputed with 16 scatter matmuls into PSUM-resident [u, (b,t)] tables.
Motor trajectory is collected in SBUF and hit with one output matmul (output
affine + dense layer folded in, bias via an extra ones row).
All parameter tables ride in one packed [128, K] constant tensor -> a single
DMA/semaphore (compiler limits per-instruction sync waits).
"""

import numpy as np

MOTOR, COMMAND, INTER = 32, 64, 128
UNITS = MOTOR + COMMAND + INTER  # order: motor, command, inter
SENSORY = 64
ODE_UNFOLDS = 6
EPS = 1e-8
B_FULL, T_LEN, OUT_LEN = 32, 32, 32
N_CORES = 8
B_LOC = B_FULL // N_CORES  # 4
BT = B_LOC * T_LEN  # 128

# name -> (rows, free_elems)
_SPEC_ORDER = [
    ("sg1", INTER, B_LOC * COMMAND),
    ("sm1", INTER, B_LOC * COMMAND),
    ("W1", INTER, 2 * COMMAND),
    ("M1", 2 * COMMAND, B_LOC * COMMAND),
    ("biasc", 2 * COMMAND, 1),
    ("sg2", COMMAND, B_LOC * (COMMAND + MOTOR)),
    ("sm2", COMMAND, B_LOC * (COMMAND + MOTOR)),
    ("W2a", COMMAND, 2 * COMMAND),
    ("W2b", COMMAND, 2 * MOTOR),
    ("M2", 2 * MOTOR, B_LOC * MOTOR),
    ("biasm", 2 * MOTOR, 1),
    ("cmt_c", COMMAND, 1),
    ("cmt_m", MOTOR, 1),
    ("cmt_i", INTER, 1),
    ("glvl_i", INTER, 1),
    ("cgle_i", INTER, 1),
    ("sgs", SENSORY, 16),
    ("sms", SENSORY, 16),
    ("WsN", SENSORY, 16 * INTER),
    ("WsD", SENSORY, 16 * INTER),
    ("dwp", MOTOR + 1, OUT_LEN),
]
_SPEC_OFF = {}
_K = 0
for _n, _r, _c in _SPEC_ORDER:
    _SPEC_OFF[_n] = (_r, _K, _c)
    _K += _c

_CACHE = {}


def _hoist_embedded_waits(bir_bytes):
    """This walrus build rejects instructions with multiple embedded sync
    waits; hoist every embedded wait into a standalone EventSemaphore
    instruction placed just before it on the same engine stream."""
    import json as _json

    ctr = [0]

    def fix_block(bb):
        out = []
        for ins in bb.get("instructions", []):
            si = ins.get("sync_info")
            if si and si.get("on_wait"):
                for w in si["on_wait"]:
                    ctr[0] += 1
                    out.append({
                        "debug": ins.get("debug", 0),
                        "engine": ins["engine"],
                        "ins": [],
                        "outs": [],
                        "name": f"EVW-{ctr[0]}",
                        "opcode": "EventSemaphore",
                        "sync_info": {"on_update": [], "on_wait": [w]},
                    })
                si["on_wait"] = []
            out.append(ins)
        bb["instructions"] = out
        for sub in bb.get("blocks", []) or []:
            fix_block(sub)

    m = _json.loads(bir_bytes)
    for fn in m["functions"]:
        for bb in fn.get("blocks", []):
            fix_block(bb)
    return _json.dumps(m).encode()


def _build_program():
    from contextlib import ExitStack

    import concourse.bass as bass
    import concourse.tile as tile
    import concourse.mybir as mybir

    F = mybir.dt.float32
    A = mybir.AluOpType
    ACTF = mybir.ActivationFunctionType

    nc = bass.Bass("TRN2", target_bir_lowering=False, debug=False)

    xT = nc.dram_tensor("xT", [SENSORY, BT], F, kind="ExternalInput").ap()
    CT = nc.dram_tensor("CT", [128, _K], F, kind="ExternalInput").ap()
    out_d = nc.dram_tensor(
        "out", [B_LOC, T_LEN, OUT_LEN], F, kind="ExternalOutput"
    ).ap()

    with tile.TileContext(nc) as tc, ExitStack() as ctx:
        const = ctx.enter_context(tc.tile_pool(name="const", bufs=1))
        state = ctx.enter_context(tc.tile_pool(name="state", bufs=1))
        prep = ctx.enter_context(tc.tile_pool(name="prep", bufs=2))
        zp = ctx.enter_context(tc.tile_pool(name="zp", bufs=3))
        sp = ctx.enter_context(tc.tile_pool(name="sp", bufs=3))
        ndp = ctx.enter_context(tc.tile_pool(name="ndp", bufs=2))
        tp = ctx.enter_context(tc.tile_pool(name="tp", bufs=3))
        ps_sens = ctx.enter_context(tc.tile_pool(name="ps_sens", bufs=1, space="PSUM"))
        ps_work = ctx.enter_context(tc.tile_pool(name="ps_work", bufs=2, space="PSUM"))
        ps_out = ctx.enter_context(tc.tile_pool(name="ps_out", bufs=1, space="PSUM"))

        ct = const.tile([128, _K], F, tag="ct")
        nc.gpsimd.dma_start(out=ct, in_=CT)
        xT_s = const.tile([SENSORY, BT], F, tag="xT")
        nc.gpsimd.dma_start(out=xT_s, in_=xT)
        # collapse the many DMA-queue completion sems into one sync point:
        # engine instructions have a small embedded-wait budget.
        tc.strict_bb_all_engine_barrier()

        def cs(name):
            r, c0, cn = _SPEC_OFF[name]
            return ct[0:r, c0 : c0 + cn]

        sg1_s = cs("sg1").rearrange("p (b j) -> p b j", b=B_LOC)
        sm1_s = cs("sm1").rearrange("p (b j) -> p b j", b=B_LOC)
        W1_s = cs("W1")
        M1_s = cs("M1").rearrange("p (b j) -> p b j", b=B_LOC)
        biasc_s = cs("biasc")
        sg2_s = cs("sg2").rearrange("p (b j) -> p b j", b=B_LOC)
        sm2_s = cs("sm2").rearrange("p (b j) -> p b j", b=B_LOC)
        W2a_s = cs("W2a")
        W2b_s = cs("W2b")
        M2_s = cs("M2").rearrange("p (b j) -> p b j", b=B_LOC)
        biasm_s = cs("biasm")
        cmt_c_s = cs("cmt_c")
        cmt_m_s = cs("cmt_m")
        cmt_i_s = cs("cmt_i")
        glvl_i_s = cs("glvl_i")
        cgle_i_s = cs("cgle_i")
        sgs_s = cs("sgs")
        sms_s = cs("sms")
        WsN_s = cs("WsN").rearrange("p (o u) -> p o u", o=16)
        WsD_s = cs("WsD").rearrange("p (o u) -> p o u", o=16)
        dwp_s = cs("dwp")

        # ---- state ----
        v_int = state.tile([INTER, B_LOC], F)
        v_cmd = state.tile([COMMAND, B_LOC], F)
        v_mot = state.tile([MOTOR, B_LOC], F)
        vma = state.tile([MOTOR + 1, BT], F)  # motor traj + ones row
        nc.vector.memset(v_int, 0.0)
        nc.vector.memset(v_cmd, 0.0)
        nc.vector.memset(v_mot, 0.0)
        nc.vector.memset(vma[MOTOR : MOTOR + 1, :], 1.0)

        # ---- sensory precompute: NS/DS [inter, (b,t)] in PSUM ----
        zs = zp.tile([SENSORY, 16, BT], F, tag="zs")
        x_bc = xT_s[:, :].unsqueeze(1).broadcast_to([SENSORY, 16, BT])
        sgs_bc = sgs_s.unsqueeze(2).broadcast_to([SENSORY, 16, BT])
        sms_bc = sms_s.unsqueeze(2).broadcast_to([SENSORY, 16, BT])
        nc.vector.tensor_mul(zs, x_bc, sgs_bc)
        nc.vector.tensor_sub(zs, zs, sms_bc)
        sigs = sp.tile([SENSORY, 16, BT], F, tag="sigs")
        nc.scalar.activation(sigs, zs, ACTF.Sigmoid)
        NS = ps_sens.tile([INTER, BT], F, tag="NS")
        DS = ps_sens.tile([INTER, BT], F, tag="DS")
        for o in range(16):
            nc.tensor.matmul(
                NS, lhsT=WsN_s[:, o, :], rhs=sigs[:, o, :],
                start=(o == 0), stop=(o == 15),
            )
        for o in range(16):
            nc.tensor.matmul(
                DS, lhsT=WsD_s[:, o, :], rhs=sigs[:, o, :],
                start=(o == 0), stop=(o == 15),
            )
        NS3 = NS.rearrange("p (b t) -> p t b", b=B_LOC)
        DS3 = DS.rearrange("p (b t) -> p t b", b=B_LOC)

        # ---- the scan ----
        for t in range(T_LEN):
            den_i = prep.tile([INTER, B_LOC], F, tag="den_i")
            nc.vector.tensor_scalar_add(den_i, DS3[:, t, :], cgle_i_s)
            rD = prep.tile([INTER, B_LOC], F, tag="rD")
            nc.vector.reciprocal(rD, den_i)
            a_i = prep.tile([INTER, B_LOC], F, tag="a_i")
            nc.vector.tensor_scalar_mul(a_i, rD, cmt_i_s)
            c_i = prep.tile([INTER, B_LOC], F, tag="c_i")
            nc.vector.scalar_tensor_tensor(
                c_i, in0=NS3[:, t, :], scalar=glvl_i_s, in1=rD,
                op0=A.add, op1=A.mult,
            )

            for u in range(ODE_UNFOLDS):
                z1 = zp.tile([INTER, B_LOC, COMMAND], F, tag="z1")
                vb1 = v_int[:, :].unsqueeze(2).broadcast_to([INTER, B_LOC, COMMAND])
                nc.vector.tensor_mul(z1, sg1_s, vb1)
                nc.vector.tensor_sub(z1, z1, sm1_s)
                sig1 = sp.tile([INTER, B_LOC, COMMAND], F, tag="sig1")
                nc.scalar.activation(sig1, z1, ACTF.Sigmoid)

                z2 = zp.tile([COMMAND, B_LOC, COMMAND + MOTOR], F, tag="z2")
                vb2 = v_cmd[:, :].unsqueeze(2).broadcast_to(
                    [COMMAND, B_LOC, COMMAND + MOTOR]
                )
                nc.vector.tensor_mul(z2, sg2_s, vb2)
                nc.vector.tensor_sub(z2, z2, sm2_s)
                sig2 = sp.tile([COMMAND, B_LOC, COMMAND + MOTOR], F, tag="sig2")
                nc.scalar.activation(sig2, z2, ACTF.Sigmoid)

                ps_c = ps_work.tile([2 * COMMAND, B_LOC * COMMAND], F, tag="ps_c")
                nc.tensor.matmul(
                    ps_c, lhsT=W1_s,
                    rhs=sig1.rearrange("p b j -> p (b j)"),
                    start=True, stop=False,
                )
                nc.tensor.matmul(
                    ps_c, lhsT=W2a_s, rhs=sig2[:, :, 0:COMMAND],
                    start=False, stop=True,
                )
                ps_m = ps_work.tile([2 * MOTOR, B_LOC * MOTOR], F, tag="ps_m")
                nc.tensor.matmul(
                    ps_m, lhsT=W2b_s,
                    rhs=sig2[:, :, COMMAND : COMMAND + MOTOR],
                    start=True, stop=True,
                )

                dc = ndp.tile([2 * COMMAND, B_LOC, COMMAND], F, tag="dc")
                nc.vector.scalar_tensor_tensor(
                    dc, in0=ps_c.rearrange("p (b j) -> p b j", b=B_LOC),
                    scalar=biasc_s, in1=M1_s, op0=A.add, op1=A.mult,
                )
                nd_c = ndp.tile([2 * COMMAND, B_LOC], F, tag="nd_c")
                nc.vector.tensor_reduce(
                    nd_c, dc, axis=mybir.AxisListType.X, op=A.add
                )
                dm = ndp.tile([2 * MOTOR, B_LOC, MOTOR], F, tag="dm")
                nc.vector.scalar_tensor_tensor(
                    dm, in0=ps_m.rearrange("p (b j) -> p b j", b=B_LOC),
                    scalar=biasm_s, in1=M2_s, op0=A.add, op1=A.mult,
                )
                nd_m = ndp.tile([2 * MOTOR, B_LOC], F, tag="nd_m")
                nc.vector.tensor_reduce(
                    nd_m, dm, axis=mybir.AxisListType.X, op=A.add
                )

                numc = tp.tile([COMMAND, B_LOC], F, tag="numc")
                nc.vector.scalar_tensor_tensor(
                    numc, in0=v_cmd, scalar=cmt_c_s, in1=nd_c[0:COMMAND, :],
                    op0=A.mult, op1=A.add,
                )
                rc = tp.tile([COMMAND, B_LOC], F, tag="rc")
                nc.vector.reciprocal(rc, nd_c[COMMAND : 2 * COMMAND, :])
                numm = tp.tile([MOTOR, B_LOC], F, tag="numm")
                nc.vector.scalar_tensor_tensor(
                    numm, in0=v_mot, scalar=cmt_m_s, in1=nd_m[0:MOTOR, :],
                    op0=A.mult, op1=A.add,
                )
                rm = tp.tile([MOTOR, B_LOC], F, tag="rm")
                nc.vector.reciprocal(rm, nd_m[MOTOR : 2 * MOTOR, :])

                ti = tp.tile([INTER, B_LOC], F, tag="ti")
                nc.vector.tensor_mul(ti, v_int, a_i)
                nc.vector.tensor_add(v_int, ti, c_i)
                nc.vector.tensor_mul(v_cmd, numc, rc)
                if u == ODE_UNFOLDS - 1:
                    vslot = vma[0:MOTOR, :].rearrange("p (b t) -> p t b", b=B_LOC)
                    nc.vector.tensor_mul(vslot[:, t, :], numm, rm)
                    nc.scalar.copy(v_mot, vslot[:, t, :])
                else:
                    nc.vector.tensor_mul(v_mot, numm, rm)

        # ---- output: [bt, out] = vma.T @ dwp (bias via ones row) ----
        out_ps = ps_out.tile([BT, OUT_LEN], F)
        nc.tensor.matmul(out_ps, lhsT=vma, rhs=dwp_s, start=True, stop=True)
        out_sb = const.tile([BT, OUT_LEN], F, tag="out_sb")
        nc.scalar.copy(out_sb, out_ps)
        nc.sync.dma_start(
            out=out_d.rearrange("b t o -> (b t) o"), in_=out_sb
        )

    orig_json = nc.to_json_bytes
    nc.to_json_bytes = lambda: _hoist_embedded_waits(orig_json())
    return nc


def _prep_tables(inp):
    """Host-side parameter/layout prep (pure transposes/products of params)."""
    g = {k: np.asarray(v, np.float32) for k, v in inp.items()}
    M, C, I = MOTOR, COMMAND, INTER
    sl_m = slice(0, M)
    sl_c = slice(M, M + C)
    sl_i = slice(M + C, UNITS)

    sigma, mu, w = g["sigma"], g["mu"], g["w"]
    erev, mask = g["erev"], g["mask"]
    smu = sigma * mu
    wer, wma = w * erev, w * mask
    cmt = g["cm"] * float(ODE_UNFOLDS)
    gl, vl = g["gleak"], g["vleak"]
    glvl = gl * vl
    cge = cmt + gl + EPS

    tb = {}
    rep = lambda a: np.repeat(a[:, None, :], B_LOC, axis=1).reshape(a.shape[0], -1)
    tb["sg1"] = rep(sigma[sl_i, sl_c])
    tb["sm1"] = rep(smu[sl_i, sl_c])
    tb["W1"] = np.concatenate([wer[sl_i, sl_c], wma[sl_i, sl_c]], axis=1)
    eyeC = np.eye(C, dtype=np.float32)
    tb["M1"] = rep(np.concatenate([eyeC, eyeC], 0))
    tb["biasc"] = np.concatenate([glvl[sl_c], cge[sl_c]])[:, None]

    tb["sg2"] = rep(np.concatenate([sigma[sl_c, sl_c], sigma[sl_c, sl_m]], axis=1))
    tb["sm2"] = rep(np.concatenate([smu[sl_c, sl_c], smu[sl_c, sl_m]], axis=1))
    tb["W2a"] = np.concatenate([wer[sl_c, sl_c], wma[sl_c, sl_c]], axis=1)
    tb["W2b"] = np.concatenate([wer[sl_c, sl_m], wma[sl_c, sl_m]], axis=1)
    eyeM = np.eye(M, dtype=np.float32)
    tb["M2"] = rep(np.concatenate([eyeM, eyeM], 0))
    tb["biasm"] = np.concatenate([glvl[sl_m], cge[sl_m]])[:, None]

    tb["cmt_c"] = cmt[sl_c][:, None]
    tb["cmt_m"] = cmt[sl_m][:, None]
    tb["cmt_i"] = cmt[sl_i][:, None]
    tb["glvl_i"] = glvl[sl_i][:, None]
    tb["cgle_i"] = cge[sl_i][:, None]

    # sensory fan-out (16 targets per sensory unit, all inter)
    smask, serev = g["sensory_mask"], g["sensory_erev"]
    ssig, smu_s, sw = g["sensory_sigma"], g["sensory_mu"], g["sensory_w"]
    iw, ib = g["input_w"], g["input_b"]
    sgs = np.zeros((SENSORY, 16), np.float32)
    sms = np.zeros((SENSORY, 16), np.float32)
    WsN = np.zeros((SENSORY, 16, I), np.float32)
    WsD = np.zeros((SENSORY, 16, I), np.float32)
    for s in range(SENSORY):
        tgt = np.nonzero(smask[s])[0]
        assert len(tgt) == 16 and tgt.min() >= M + C
        for o, u in enumerate(tgt):
            ul = u - (M + C)
            sgs[s, o] = ssig[s, u] * iw[s]
            sms[s, o] = ssig[s, u] * (smu_s[s, u] - ib[s])
            WsN[s, o, ul] = sw[s, u] * serev[s, u]
            WsD[s, o, ul] = sw[s, u] * smask[s, u]
    tb["sgs"], tb["sms"] = sgs, sms
    tb["WsN"] = WsN.reshape(SENSORY, -1)
    tb["WsD"] = WsD.reshape(SENSORY, -1)

    ow, ob = g["output_w"], g["output_b"]
    dw, db = g["dense_w"], g["dense_b"]
    dwp = np.zeros((M + 1, OUT_LEN), np.float32)
    dwp[:M] = ow[:, None] * dw
    dwp[M] = db + ob @ dw
    tb["dwp"] = dwp

    CTa = np.zeros((128, _K), np.float32)
    for n, r, c in _SPEC_ORDER:
        _, c0, cn = _SPEC_OFF[n]
        a = tb[n]
        assert a.shape == (r, cn), (n, a.shape, (r, cn))
        CTa[:r, c0 : c0 + cn] = a
    return CTa, g


def kernel(**inputs):
    from concourse.bass_utils import run_bass_kernel_spmd

    if "nc" not in _CACHE:
        _CACHE["nc"] = _build_program()
    nc = _CACHE["nc"]

    CTa, g = _prep_tables(inputs)
    x = g["inputs"]  # [B, T, S]
    in_maps = []
    for c in range(N_CORES):
        xc = x[c * B_LOC : (c + 1) * B_LOC]  # [4, T, S]
        xTc = np.ascontiguousarray(
            np.transpose(xc, (2, 0, 1)).reshape(SENSORY, BT)
        )
        in_maps.append({"xT": xTc, "CT": CTa})

    res = run_bass_kernel_spmd(nc, in_maps, list(range(N_CORES)))
    out = np.concatenate([res.results[c]["out"] for c in range(N_CORES)], axis=0)
    return out.astype(np.float32)


if __name__ == "__main__":
    import reference

    inp = {k: np.asarray(v) for k, v in reference.setup_inputs().items()}
    got = kernel(**inp)
    want = np.asarray(reference.reference(**reference.setup_inputs()))
    err = np.abs(got - want).max() / (np.abs(want).max() + 1e-12)
    print("Relative error:", err)



# revision 20
# speedup vs baseline: 2.0560x; 2.0560x over previous
"""Trainium2 Bass kernel for a Neural Circuit Policies (LTC) cell.

Strategy (v2): data-parallel over batch (32 -> 8 cores x 4). Per core the
T=32 x 6-unfold scan runs fully unrolled with a 2-way batch-pair interleave
to hide cross-engine latency.

Key structure:
- Inter neurons receive no recurrent synapses, so their whole 192-step
  trajectory is an affine recurrence v' = A_t v + C_t with per-step (t)
  coefficients from the sensory tables. It is precomputed OUTSIDE the scan
  (closed-form powers of A_t expand the 6 unfolds per step), removing inter
  from the serial chain.
- Recurrent reductions use per-synapse-slot scatter matmuls (PE issue is
  ~3.5ns/matmul): inter->cmd uses 16 slots with the erev sign folded into a
  (num,den) rhs pair (premultiplied on gpsimd); cmd->cmd/motor uses 17 slots
  with separate num/den matmuls (sign lives in the lhsT). PSUM receives
  num/den [96, b] directly -- no diagonal extraction.
- Leak/eps biases ride a constant-sigmoid slot (z=40 -> sig=1) of a
  low-degree cmd unit.
- cmd+motor state lives in a [97, b, 33] history tile (ones row for the
  output bias); each unfold's divide writes column ceil(k/6), so the motor
  trajectory needed by the output matmul materializes with zero extra ops.
- Output: per half one matmul (hist motor rows + ones row as lhsT) x packed
  dense weights.
"""

import numpy as np

MOTOR, COMMAND, INTER = 32, 64, 128
UNITS = MOTOR + COMMAND + INTER  # order: motor, command, inter
SENSORY = 64
ODE_UNFOLDS = 6
EPS = 1e-8
B_FULL, T_LEN, OUT_LEN = 32, 32, 32
N_CORES = 8
B_LOC = B_FULL // N_CORES  # 4
BT = B_LOC * T_LEN  # 128
NH = 2  # interleaved halves
BH = B_LOC // NH  # batch per half
NPOST = COMMAND + MOTOR  # 96
S1 = 16  # inter out-slots (exact fanout)
S2 = 17  # cmd out-slots (max out-degree, slot also carries the bias)

# name -> (rows, free_elems); all f32, packed into one [128, K] DMA
_SPEC_ORDER = [
    ("sg1", INTER, S1 * B_LOC),        # [p, s, b] replicated over b
    ("sm1", INTER, S1 * B_LOC),
    ("W1n", INTER, S1 * NPOST),        # signed inter lhsT (w*erev)
    ("W1d", INTER, S1 * NPOST),        # unsigned inter lhsT (w*mask)
    ("sg2", COMMAND, S2 * BH),         # [c, s, bh] replicated over bh
    ("sm2", COMMAND, S2 * BH),
    ("W2n", COMMAND, S2 * NPOST),      # signed lhsT (w*erev [+ bias row])
    ("W2d", COMMAND, S2 * NPOST),      # unsigned lhsT (w*mask [+ bias row])
    ("CMTD", NPOST, NPOST),            # diag(cmt) lhsT folding cmt*v into ps
    ("cmt_cm", NPOST, 1),
    ("cmt_i", INTER, 1),
    ("glvl_i", INTER, 1),
    ("cgle_i", INTER, 1),
    ("sgs", SENSORY, 16),
    ("sms", SENSORY, 16),
    ("WsN", SENSORY, 16 * INTER),
    ("WsD", SENSORY, 16 * INTER),
    ("dwp", MOTOR + 1, OUT_LEN),
]
_SPEC_OFF = {}
_K = 0
for _n, _r, _c in _SPEC_ORDER:
    _SPEC_OFF[_n] = (_r, _K, _c)
    _K += _c

_CACHE = {}


def _hoist_embedded_waits(bir_bytes):
    """This walrus build rejects instructions with multiple embedded sync
    waits; hoist every embedded wait into a standalone EventSemaphore
    instruction placed just before it on the same engine stream."""
    import json as _json

    ctr = [0]

    def fix_block(bb):
        out = []
        for ins in bb.get("instructions", []):
            si = ins.get("sync_info")
            if si and si.get("on_wait"):
                for w in si["on_wait"]:
                    ctr[0] += 1
                    out.append({
                        "debug": ins.get("debug", 0),
                        "engine": ins["engine"],
                        "ins": [],
                        "outs": [],
                        "name": f"EVW-{ctr[0]}",
                        "opcode": "EventSemaphore",
                        "sync_info": {"on_update": [], "on_wait": [w]},
                    })
                si["on_wait"] = []
            out.append(ins)
        bb["instructions"] = out
        for sub in bb.get("blocks", []) or []:
            fix_block(sub)

    m = _json.loads(bir_bytes)
    for fn in m["functions"]:
        for bb in fn.get("blocks", []):
            fix_block(bb)
    return _json.dumps(m).encode()


def _build_program():
    from contextlib import ExitStack

    import concourse.bass as bass
    import concourse.tile as tile
    import concourse.mybir as mybir

    F = mybir.dt.float32
    A = mybir.AluOpType
    ACTF = mybir.ActivationFunctionType

    nc = bass.Bass("TRN2", target_bir_lowering=False, debug=False)

    xT = nc.dram_tensor("xT", [SENSORY, BT], F, kind="ExternalInput").ap()
    CT = nc.dram_tensor("CT", [128, _K], F, kind="ExternalInput").ap()
    out_d = nc.dram_tensor(
        "out", [B_LOC, T_LEN, OUT_LEN], F, kind="ExternalOutput"
    ).ap()

    with tile.TileContext(nc) as tc, ExitStack() as ctx:
        const = ctx.enter_context(tc.tile_pool(name="const", bufs=1))
        state = ctx.enter_context(tc.tile_pool(name="state", bufs=1))
        pre = ctx.enter_context(tc.tile_pool(name="pre", bufs=2))
        zp = ctx.enter_context(tc.tile_pool(name="zp", bufs=3))
        sp = ctx.enter_context(tc.tile_pool(name="sp", bufs=3))
        up = ctx.enter_context(tc.tile_pool(name="up", bufs=3))
        ps_sens = ctx.enter_context(tc.tile_pool(name="ps_sens", bufs=1, space="PSUM"))
        ps_work = ctx.enter_context(tc.tile_pool(name="ps_work", bufs=2, space="PSUM"))
        ps_out = ctx.enter_context(tc.tile_pool(name="ps_out", bufs=1, space="PSUM"))

        ct = const.tile([128, _K], F, tag="ct")
        nc.gpsimd.dma_start(out=ct, in_=CT)
        xT_s = const.tile([SENSORY, BT], F, tag="xT")
        nc.gpsimd.dma_start(out=xT_s, in_=xT)
        tc.strict_bb_all_engine_barrier()

        def cs(name):
            r, c0, cn = _SPEC_OFF[name]
            if name == "dwp":  # aligned with hist motor rows for the out matmul
                return ct[COMMAND : COMMAND + r, c0 : c0 + cn]
            return ct[0:r, c0 : c0 + cn]

        sg1_s = cs("sg1").rearrange("p (s b) -> p s b", s=S1)
        sm1_s = cs("sm1").rearrange("p (s b) -> p s b", s=S1)
        W1n_s = cs("W1n").rearrange("p (s q) -> p s q", s=S1)
        W1d_s = cs("W1d").rearrange("p (s q) -> p s q", s=S1)
        sg2_s = cs("sg2").rearrange("p (s b) -> p s b", s=S2)
        sm2_s = cs("sm2").rearrange("p (s b) -> p s b", s=S2)
        W2n_s = cs("W2n").rearrange("p (s q) -> p s q", s=S2)
        W2d_s = cs("W2d").rearrange("p (s q) -> p s q", s=S2)
        CMTD_s = cs("CMTD")
        cmt_cm_s = cs("cmt_cm")
        cmt_i_s = cs("cmt_i")
        glvl_i_s = cs("glvl_i")
        cgle_i_s = cs("cgle_i")
        sgs_s = cs("sgs")
        sms_s = cs("sms")
        WsN_s = cs("WsN").rearrange("p (o u) -> p o u", o=16)
        WsD_s = cs("WsD").rearrange("p (o u) -> p o u", o=16)
        dwp_s = cs("dwp")

        # ---- sensory precompute: NS/DS [inter, (b,t)] in PSUM ----
        zs = zp.tile([SENSORY, 16, BT], F, tag="zs")
        x_bc = xT_s[:, :].unsqueeze(1).broadcast_to([SENSORY, 16, BT])
        sgs_bc = sgs_s.unsqueeze(2).broadcast_to([SENSORY, 16, BT])
        sms_bc = sms_s.unsqueeze(2).broadcast_to([SENSORY, 16, BT])
        nc.vector.tensor_mul(zs, x_bc, sgs_bc)
        nc.vector.tensor_sub(zs, zs, sms_bc)
        sigs = sp.tile([SENSORY, 16, BT], F, tag="sigs")
        nc.scalar.activation(sigs, zs, ACTF.Sigmoid)
        NS = ps_sens.tile([INTER, BT], F, tag="NS")
        DS = ps_sens.tile([INTER, BT], F, tag="DS")
        for o in range(16):
            nc.tensor.matmul(
                NS, lhsT=WsN_s[:, o, :], rhs=sigs[:, o, :],
                start=(o == 0), stop=(o == 15),
            )
        for o in range(16):
            nc.tensor.matmul(
                DS, lhsT=WsD_s[:, o, :], rhs=sigs[:, o, :],
                start=(o == 0), stop=(o == 15),
            )

        # ---- inter trajectory precompute ----
        # per (p,b,t): den = DS + cge; a = cmt_i/den; c = (NS+glvl)/den
        # v(t,u) = a_t^u v(t,0) + (sum_{k<u} a_t^k) c_t; v(t+1,0) from u=6.
        aP = pre.tile([INTER, B_LOC, T_LEN], F, tag="aP")
        cP = pre.tile([INTER, B_LOC, T_LEN], F, tag="cP")
        den_i = pre.tile([INTER, B_LOC, T_LEN], F, tag="den_i")
        NS3 = NS.rearrange("p (b t) -> p b t", b=B_LOC)
        DS3 = DS.rearrange("p (b t) -> p b t", b=B_LOC)
        nc.vector.tensor_scalar_add(den_i, DS3, cgle_i_s)
        rD = pre.tile([INTER, B_LOC, T_LEN], F, tag="rD")
        nc.vector.reciprocal(rD, den_i)
        nc.vector.tensor_scalar_mul(aP, rD, cmt_i_s)
        nc.vector.scalar_tensor_tensor(
            cP, in0=NS3, scalar=glvl_i_s, in1=rD, op0=A.add, op1=A.mult
        )
        # powers a^u (u=0..6) and csum_u = (sum_{k<u} a^k) * c
        apw = pre.tile([INTER, B_LOC, T_LEN, ODE_UNFOLDS + 1], F, tag="apw")
        csm = pre.tile([INTER, B_LOC, T_LEN, ODE_UNFOLDS + 1], F, tag="csm")
        ssum = pre.tile([INTER, B_LOC, T_LEN], F, tag="ssum")
        nc.vector.memset(apw[:, :, :, 0], 1.0)
        nc.vector.memset(csm[:, :, :, 0], 0.0)
        nc.vector.memset(ssum, 1.0)  # S_1 = a^0
        for u in range(1, ODE_UNFOLDS + 1):
            # v(t,u) = a^u v(t,0) + S_u c,  S_u = sum_{j<u} a^j
            nc.vector.tensor_mul(apw[:, :, :, u], apw[:, :, :, u - 1], aP)
            nc.vector.tensor_mul(csm[:, :, :, u], ssum, cP)
            if u < ODE_UNFOLDS:
                nc.vector.tensor_add(ssum, ssum, apw[:, :, :, u])
        v0 = pre.tile([INTER, B_LOC, T_LEN + 1], F, tag="v0")
        nc.vector.memset(v0[:, :, 0], 0.0)
        for t in range(T_LEN):
            nc.vector.tensor_mul(
                v0[:, :, t + 1], v0[:, :, t], apw[:, :, t, ODE_UNFOLDS]
            )
            nc.vector.tensor_add(
                v0[:, :, t + 1], v0[:, :, t + 1], csm[:, :, t, ODE_UNFOLDS]
            )
        VI = pre.tile([INTER, B_LOC, T_LEN, ODE_UNFOLDS], F, tag="VI")
        v0bc = v0[:, :, 0:T_LEN].unsqueeze(3).broadcast_to(
            [INTER, B_LOC, T_LEN, ODE_UNFOLDS]
        )
        nc.vector.tensor_mul(VI, apw[:, :, :, 0:ODE_UNFOLDS], v0bc)
        nc.vector.tensor_add(VI, VI, csm[:, :, :, 0:ODE_UNFOLDS])

        zlhs = const.tile([INTER, NPOST], F, tag="zlhs")
        nc.vector.memset(zlhs, 0.0)

        # ---- state: hist [97, BH, 33] per half (cmd 0:64, motor 64:96, ones 96)
        hists = []
        for h in range(NH):
            hh = state.tile([NPOST + 1, BH, T_LEN + 1], F, tag=f"hist{h}")
            nc.vector.memset(hh, 0.0)
            nc.vector.memset(hh[NPOST : NPOST + 1, :, :], 1.0)
            hists.append(hh)

        # ---- main scan ----
        n_steps = T_LEN * ODE_UNFOLDS
        ps_prev = [None, None]

        for k in range(n_steps):
            t = k // ODE_UNFOLDS
            u = k % ODE_UNFOLDS
            # state s_j lives in col (j+5)//6 (s_0 = col 0 zeros); iteration k
            # first computes s_k from s_{k-1} + ps(k-1), then z2 reads s_k.
            rdcol = (k + 4) // 6  # col of s_{k-1}
            scol = (k + 5) // 6  # col of s_k (write target + z2 source)

            # 4b-wide inter feed, batched per t (all 6 unfolds; off-chain)
            if u == 0:
                z1 = zp.tile([INTER, ODE_UNFOLDS, S1, B_LOC], F, tag="z1")
                vi_bc = (
                    VI[:, :, t, :]
                    .rearrange("p b u -> p u b")
                    .unsqueeze(2)
                    .broadcast_to([INTER, ODE_UNFOLDS, S1, B_LOC])
                )
                sg1_bc = sg1_s.unsqueeze(1).broadcast_to(
                    [INTER, ODE_UNFOLDS, S1, B_LOC]
                )
                sm1_bc = sm1_s.unsqueeze(1).broadcast_to(
                    [INTER, ODE_UNFOLDS, S1, B_LOC]
                )
                nc.vector.tensor_tensor(z1, sg1_bc, vi_bc, op=A.mult)
                nc.vector.tensor_tensor(z1, z1, sm1_bc, op=A.subtract)
                sig1_t = sp.tile([INTER, ODE_UNFOLDS, S1, B_LOC], F, tag="sig1")
                nc.scalar.activation(sig1_t, z1, ACTF.Sigmoid)
            sig1 = sig1_t[:, u]

            # state update s_k = ps_n * (1/ps_d); two ops because a DVE
            # instruction may read only one PSUM operand. The reciprocal
            # depends only on the den side (finishes first), so it hides.
            if k > 0:
                rds = []
                for h in range(NH):
                    rd = up.tile([NPOST, BH], F, tag=f"rd{h}")
                    nc.vector.reciprocal(rd, ps_prev[h][:, 1, :])
                    rds.append(rd)
                for h in range(NH):
                    nc.vector.tensor_tensor(
                        hists[h][0:NPOST, :, scol], ps_prev[h][:, 0, :],
                        rds[h], op=A.mult,
                    )
            z2s_, sig2s_ = [], []
            for h in range(NH):
                z2 = zp.tile([COMMAND, S2, BH], F, tag=f"z2{h}")
                vcb = hists[h][0:COMMAND, :, scol].unsqueeze(1).broadcast_to(
                    [COMMAND, S2, BH]
                )
                nc.vector.tensor_tensor(z2, sg2_s, vcb, op=A.mult)
                z2s_.append(z2)
            for h in range(NH):
                nc.vector.tensor_tensor(z2s_[h], z2s_[h], sm2_s, op=A.subtract)
            for h in range(NH):
                sig2 = sp.tile([COMMAND, S2, BH], F, tag=f"sig2{h}")
                nc.scalar.activation(sig2, z2s_[h], ACTF.Sigmoid)
                sig2s_.append(sig2)

            for h in range(NH):
                hh = hists[h]
                bsl = slice(h * BH, (h + 1) * BH)
                sig2 = sig2s_[h]
                ps2 = ps_work.tile([NPOST, 2, BH], F, tag=f"ps{h}")
                # one full-tile zeroing matmul opens the group (start=True on
                # partial regions would be two opens); then pure accumulation.
                nc.tensor.matmul(
                    ps2, lhsT=zlhs, rhs=sig1[:, 0, bsl].unsqueeze(1)
                    .broadcast_to([INTER, 2, BH]),
                    start=True, stop=False, skip_group_check=True,
                )
                # den side first (ready earlier), num side last gates the div
                for s in range(S1):
                    nc.tensor.matmul(
                        ps2[:, 1, :], lhsT=W1d_s[:, s, :], rhs=sig1[:, s, bsl],
                        start=False, stop=False, skip_group_check=True,
                    )
                for s in range(S1):
                    nc.tensor.matmul(
                        ps2[:, 0, :], lhsT=W1n_s[:, s, :], rhs=sig1[:, s, bsl],
                        start=False, stop=False, skip_group_check=True,
                    )
                nc.tensor.matmul(
                    ps2[:, 0, :], lhsT=CMTD_s, rhs=hh[0:NPOST, :, scol],
                    start=False, stop=False, skip_group_check=True,
                )
                for s in range(S2):
                    nc.tensor.matmul(
                        ps2[:, 1, :], lhsT=W2d_s[:, s, :], rhs=sig2[:, s, :],
                        start=False, stop=False, skip_group_check=True,
                    )
                for s in range(S2):
                    nc.tensor.matmul(
                        ps2[:, 0, :], lhsT=W2n_s[:, s, :], rhs=sig2[:, s, :],
                        start=False, stop=(s == S2 - 1), skip_group_check=True,
                    )
                ps_prev[h] = ps2

        # final state update (k = n_steps): s_192 from s_191
        k = n_steps
        for h in range(NH):
            rd = up.tile([NPOST, BH], F, tag=f"rd{h}")
            nc.vector.reciprocal(rd, ps_prev[h][:, 1, :])
            nc.vector.tensor_tensor(
                hists[h][0:NPOST, :, (k + 5) // 6], ps_prev[h][:, 0, :],
                rd, op=A.mult,
            )

        # ---- output: per half out[(bh,t), o] = hist[64:97].T @ dwp ----
        od2 = out_d.rearrange("b t o -> (b t) o")
        for i in range(2):
            ps_o = ps_out.tile([2 * T_LEN, OUT_LEN], F, tag=f"po{i}")
            for j in range(2):
                bi = 2 * i + j  # global batch index
                h, b = bi // BH, bi % BH
                # rows 64:97 = motor (64:96) + ones (96); cols 1:33
                lh = hists[h][COMMAND:, b, 1 : T_LEN + 1]
                nc.tensor.matmul(
                    ps_o[j * T_LEN : (j + 1) * T_LEN, :], lhsT=lh, rhs=dwp_s,
                    start=True, stop=True, skip_group_check=True,
                )
            sb_o = const.tile([2 * T_LEN, OUT_LEN], F, tag=f"sbo{i}")
            nc.scalar.copy(sb_o, ps_o)
            nc.sync.dma_start(
                out=od2[2 * i * T_LEN : (2 * i + 2) * T_LEN, :], in_=sb_o
            )

    orig_json = nc.to_json_bytes
    nc.to_json_bytes = lambda: _hoist_embedded_waits(orig_json())
    return nc


def _prep_tables(inp):
    """Host-side parameter/layout prep (pure transposes/products of params)."""
    g = {k: np.asarray(v, np.float32) for k, v in inp.items()}
    M, C, I = MOTOR, COMMAND, INTER
    sl_m = slice(0, M)
    sl_c = slice(M, M + C)
    sl_i = slice(M + C, UNITS)

    sigma, mu, w = g["sigma"], g["mu"], g["w"]
    erev, mask = g["erev"], g["mask"]
    cmt = g["cm"] * float(ODE_UNFOLDS)
    gl, vl = g["gleak"], g["vleak"]
    glvl = gl * vl
    cge = cmt + gl + EPS

    # post-row mapping: cmd unit j (abs M..M+C) -> row j-M; motor j -> 64+j
    def post_row(j):
        return j - M if j >= M else C + j

    glvl_cm = np.zeros(NPOST, np.float32)
    cge_cm = np.zeros(NPOST, np.float32)
    cmt_cm = np.zeros(NPOST, np.float32)
    for j in range(M + C):
        q = post_row(j)
        glvl_cm[q] = glvl[j]
        cge_cm[q] = cge[j]
        cmt_cm[q] = cmt[j]

    tb = {}
    # inter slots (pre rows sl_i; posts all cmd)
    sg1 = np.zeros((I, S1), np.float32)
    sm1 = np.zeros((I, S1), np.float32)
    W1n = np.zeros((I, S1, NPOST), np.float32)
    W1d = np.zeros((I, S1, NPOST), np.float32)
    for p in range(I):
        pre = M + C + p
        tgt = np.nonzero(mask[pre])[0]
        assert len(tgt) == S1 and tgt.min() >= M and tgt.max() < M + C
        for s, j in enumerate(tgt):
            sg1[p, s] = sigma[pre, j]
            sm1[p, s] = sigma[pre, j] * mu[pre, j]
            W1n[p, s, post_row(j)] = w[pre, j] * erev[pre, j]
            W1d[p, s, post_row(j)] = w[pre, j]
    tb["sg1"] = np.repeat(sg1[:, :, None], B_LOC, 2).reshape(I, -1)
    tb["sm1"] = np.repeat(sm1[:, :, None], B_LOC, 2).reshape(I, -1)
    tb["W1n"] = W1n.reshape(I, -1)
    tb["W1d"] = W1d.reshape(I, -1)

    # cmd slots (pre rows sl_c; posts cmd+motor); slot S2-1 may carry bias
    deg = np.array([np.count_nonzero(mask[M + c, : M + C]) for c in range(C)])
    assert deg.max() <= S2
    cmin = int(np.argmin(deg))
    assert deg[cmin] < S2
    sg2 = np.zeros((C, S2), np.float32)
    sm2 = np.zeros((C, S2), np.float32)
    W2n = np.zeros((C, S2, NPOST), np.float32)
    W2d = np.zeros((C, S2, NPOST), np.float32)
    for c in range(C):
        pre = M + c
        tgt = np.nonzero(mask[pre, : M + C])[0]
        for s, j in enumerate(tgt):
            sg2[c, s] = sigma[pre, j]
            sm2[c, s] = sigma[pre, j] * mu[pre, j]
            q = post_row(j)
            W2n[c, s, q] = w[pre, j] * erev[pre, j]
            W2d[c, s, q] = w[pre, j]
    # bias pseudo-synapse: constant sigmoid 1 on (cmin, S2-1)
    assert np.count_nonzero(W2d[cmin, S2 - 1]) == 0
    sg2[cmin, S2 - 1] = 0.0
    sm2[cmin, S2 - 1] = -40.0  # z = -sm -> sigmoid(40) = 1
    W2n[cmin, S2 - 1, :] = glvl_cm
    W2d[cmin, S2 - 1, :] = cge_cm
    tb["sg2"] = np.repeat(sg2[:, :, None], BH, 2).reshape(C, -1)
    tb["sm2"] = np.repeat(sm2[:, :, None], BH, 2).reshape(C, -1)
    tb["W2n"] = W2n.reshape(C, -1)
    tb["W2d"] = W2d.reshape(C, -1)

    tb["CMTD"] = np.diag(cmt_cm).astype(np.float32)
    tb["cmt_cm"] = cmt_cm[:, None]
    tb["cmt_i"] = cmt[sl_i][:, None]
    tb["glvl_i"] = glvl[sl_i][:, None]
    tb["cgle_i"] = cge[sl_i][:, None]

    # sensory fan-out (16 targets per sensory unit, all inter)
    smask, serev = g["sensory_mask"], g["sensory_erev"]
    ssig, smu_s, sw = g["sensory_sigma"], g["sensory_mu"], g["sensory_w"]
    iw, ib = g["input_w"], g["input_b"]
    sgs = np.zeros((SENSORY, 16), np.float32)
    sms = np.zeros((SENSORY, 16), np.float32)
    WsN = np.zeros((SENSORY, 16, I), np.float32)
    WsD = np.zeros((SENSORY, 16, I), np.float32)
    for s in range(SENSORY):
        tgt = np.nonzero(smask[s])[0]
        assert len(tgt) == 16 and tgt.min() >= M + C
        for o, uu in enumerate(tgt):
            ul = uu - (M + C)
            sgs[s, o] = ssig[s, uu] * iw[s]
            sms[s, o] = ssig[s, uu] * (smu_s[s, uu] - ib[s])
            WsN[s, o, ul] = sw[s, uu] * serev[s, uu]
            WsD[s, o, ul] = sw[s, uu] * smask[s, uu]
    tb["sgs"], tb["sms"] = sgs, sms
    tb["WsN"] = WsN.reshape(SENSORY, -1)
    tb["WsD"] = WsD.reshape(SENSORY, -1)

    # output: motor rows of hist are post_row order 64..95 = motor unit j
    ow, ob = g["output_w"], g["output_b"]
    dw, db = g["dense_w"], g["dense_b"]
    dwp = np.zeros((M + 1, OUT_LEN), np.float32)
    dwp[:M] = ow[:, None] * dw
    dwp[M] = db + ob @ dw
    tb["dwp"] = dwp

    CTa = np.zeros((128, _K), np.float32)
    for n, r, c in _SPEC_ORDER:
        _, c0, cn = _SPEC_OFF[n]
        a = tb[n]
        assert a.shape == (r, cn), (n, a.shape, (r, cn))
        r0 = COMMAND if n == "dwp" else 0  # dwp rides rows 64:97
        CTa[r0 : r0 + r, c0 : c0 + cn] = a
    return CTa, g


def kernel(**inputs):
    from concourse.bass_utils import run_bass_kernel_spmd

    if "nc" not in _CACHE:
        _CACHE["nc"] = _build_program()
    nc = _CACHE["nc"]

    CTa, g = _prep_tables(inputs)
    x = g["inputs"]  # [B, T, S]
    in_maps = []
    for c in range(N_CORES):
        xc = x[c * B_LOC : (c + 1) * B_LOC]  # [4, T, S]
        xTc = np.ascontiguousarray(
            np.transpose(xc, (2, 0, 1)).reshape(SENSORY, BT)
        )
        in_maps.append({"xT": xTc, "CT": CTa})

    res = run_bass_kernel_spmd(nc, in_maps, list(range(N_CORES)))
    out = np.concatenate([res.results[c]["out"] for c in range(N_CORES)], axis=0)
    return out.astype(np.float32)


if __name__ == "__main__":
    import reference

    inp = {k: np.asarray(v) for k, v in reference.setup_inputs().items()}
    got = kernel(**inp)
    want = np.asarray(reference.reference(**reference.setup_inputs()))
    err = np.abs(got - want).max() / (np.abs(want).max() + 1e-12)
    print("Relative error:", err)


# revision 21
# speedup vs baseline: 2.1449x; 1.0433x over previous
"""Trainium2 Bass kernel for a Neural Circuit Policies (LTC) cell.

Strategy (v2): data-parallel over batch (32 -> 8 cores x 4). Per core the
T=32 x 6-unfold scan runs fully unrolled with a 2-way batch-pair interleave
to hide cross-engine latency.

Key structure:
- Inter neurons receive no recurrent synapses, so their whole 192-step
  trajectory is an affine recurrence v' = A_t v + C_t with per-step (t)
  coefficients from the sensory tables. It is precomputed OUTSIDE the scan
  (closed-form powers of A_t expand the 6 unfolds per step), removing inter
  from the serial chain.
- Recurrent reductions use per-synapse-slot scatter matmuls (PE issue is
  ~3.5ns/matmul): inter->cmd uses 16 slots with the erev sign folded into a
  (num,den) rhs pair (premultiplied on gpsimd); cmd->cmd/motor uses 17 slots
  with separate num/den matmuls (sign lives in the lhsT). PSUM receives
  num/den [96, b] directly -- no diagonal extraction.
- Leak/eps biases ride a constant-sigmoid slot (z=40 -> sig=1) of a
  low-degree cmd unit.
- cmd+motor state lives in a [97, b, 33] history tile (ones row for the
  output bias); each unfold's divide writes column ceil(k/6), so the motor
  trajectory needed by the output matmul materializes with zero extra ops.
- Output: per half one matmul (hist motor rows + ones row as lhsT) x packed
  dense weights.
"""

import numpy as np

MOTOR, COMMAND, INTER = 32, 64, 128
UNITS = MOTOR + COMMAND + INTER  # order: motor, command, inter
SENSORY = 64
ODE_UNFOLDS = 6
EPS = 1e-8
B_FULL, T_LEN, OUT_LEN = 32, 32, 32
N_CORES = 8
B_LOC = B_FULL // N_CORES  # 4
BT = B_LOC * T_LEN  # 128
NH = 2  # interleaved halves
BH = B_LOC // NH  # batch per half
NPOST = COMMAND + MOTOR  # 96
S1 = 16  # inter out-slots (exact fanout)
S2 = 17  # cmd out-slots (max out-degree, slot also carries the bias)

# name -> (rows, free_elems); all f32, packed into one [128, K] DMA
_SPEC_ORDER = [
    ("sg1", INTER, S1 * B_LOC),        # [p, s, b] replicated over b
    ("sm1", INTER, S1 * B_LOC),
    ("W1n", INTER, S1 * NPOST),        # signed inter lhsT (w*erev)
    ("W1d", INTER, S1 * NPOST),        # unsigned inter lhsT (w*mask)
    ("sg2", COMMAND, S2 * BH),         # [c, s, bh] replicated over bh
    ("sm2", COMMAND, S2 * BH),
    ("W2n", COMMAND, S2 * NPOST),      # signed lhsT (w*erev [+ bias row])
    ("W2d", COMMAND, S2 * NPOST),      # unsigned lhsT (w*mask [+ bias row])
    ("CMTD", NPOST, NPOST),            # diag(cmt) lhsT folding cmt*v into ps
    ("cmt_cm", NPOST, 1),
    ("cmt_i", INTER, 1),
    ("glvl_i", INTER, 1),
    ("cgle_i", INTER, 1),
    ("sgs", SENSORY, 16),
    ("sms", SENSORY, 16),
    ("WsN", SENSORY, 16 * INTER),
    ("WsD", SENSORY, 16 * INTER),
    ("dwp", MOTOR + 1, OUT_LEN),
]
_SPEC_OFF = {}
_K = 0
for _n, _r, _c in _SPEC_ORDER:
    _SPEC_OFF[_n] = (_r, _K, _c)
    _K += _c

_CACHE = {}


def _hoist_embedded_waits(bir_bytes):
    """This walrus build rejects instructions with multiple embedded sync
    waits; hoist every embedded wait into a standalone EventSemaphore
    instruction placed just before it on the same engine stream."""
    import json as _json

    ctr = [0]

    def fix_block(bb):
        out = []
        for ins in bb.get("instructions", []):
            si = ins.get("sync_info")
            if si and si.get("on_wait"):
                for w in si["on_wait"]:
                    ctr[0] += 1
                    out.append({
                        "debug": ins.get("debug", 0),
                        "engine": ins["engine"],
                        "ins": [],
                        "outs": [],
                        "name": f"EVW-{ctr[0]}",
                        "opcode": "EventSemaphore",
                        "sync_info": {"on_update": [], "on_wait": [w]},
                    })
                si["on_wait"] = []
            out.append(ins)
        bb["instructions"] = out
        for sub in bb.get("blocks", []) or []:
            fix_block(sub)

    m = _json.loads(bir_bytes)
    for fn in m["functions"]:
        for bb in fn.get("blocks", []):
            fix_block(bb)
    return _json.dumps(m).encode()


def _build_program():
    from contextlib import ExitStack

    import concourse.bass as bass
    import concourse.tile as tile
    import concourse.mybir as mybir

    F = mybir.dt.float32
    A = mybir.AluOpType
    ACTF = mybir.ActivationFunctionType

    nc = bass.Bass("TRN2", target_bir_lowering=False, debug=False)

    xT = nc.dram_tensor("xT", [SENSORY, BT], F, kind="ExternalInput").ap()
    CT = nc.dram_tensor("CT", [128, _K], F, kind="ExternalInput").ap()
    out_d = nc.dram_tensor(
        "out", [B_LOC, T_LEN, OUT_LEN], F, kind="ExternalOutput"
    ).ap()

    with tile.TileContext(nc) as tc, ExitStack() as ctx:
        const = ctx.enter_context(tc.tile_pool(name="const", bufs=1))
        state = ctx.enter_context(tc.tile_pool(name="state", bufs=1))
        pre = ctx.enter_context(tc.tile_pool(name="pre", bufs=2))
        zp = ctx.enter_context(tc.tile_pool(name="zp", bufs=3))
        sp = ctx.enter_context(tc.tile_pool(name="sp", bufs=3))
        up = ctx.enter_context(tc.tile_pool(name="up", bufs=3))
        ps_sens = ctx.enter_context(tc.tile_pool(name="ps_sens", bufs=1, space="PSUM"))
        ps_work = ctx.enter_context(tc.tile_pool(name="ps_work", bufs=2, space="PSUM"))
        ps_out = ctx.enter_context(tc.tile_pool(name="ps_out", bufs=1, space="PSUM"))

        ct = const.tile([128, _K], F, tag="ct")
        nc.gpsimd.dma_start(out=ct, in_=CT)
        xT_s = const.tile([SENSORY, BT], F, tag="xT")
        nc.gpsimd.dma_start(out=xT_s, in_=xT)
        tc.strict_bb_all_engine_barrier()

        def cs(name):
            r, c0, cn = _SPEC_OFF[name]
            if name == "dwp":  # aligned with hist motor rows for the out matmul
                return ct[COMMAND : COMMAND + r, c0 : c0 + cn]
            return ct[0:r, c0 : c0 + cn]

        sg1_s = cs("sg1").rearrange("p (s b) -> p s b", s=S1)
        sm1_s = cs("sm1").rearrange("p (s b) -> p s b", s=S1)
        W1n_s = cs("W1n").rearrange("p (s q) -> p s q", s=S1)
        W1d_s = cs("W1d").rearrange("p (s q) -> p s q", s=S1)
        sg2_s = cs("sg2").rearrange("p (s b) -> p s b", s=S2)
        sm2_s = cs("sm2").rearrange("p (s b) -> p s b", s=S2)
        W2n_s = cs("W2n").rearrange("p (s q) -> p s q", s=S2)
        W2d_s = cs("W2d").rearrange("p (s q) -> p s q", s=S2)
        CMTD_s = cs("CMTD")
        cmt_cm_s = cs("cmt_cm")
        cmt_i_s = cs("cmt_i")
        glvl_i_s = cs("glvl_i")
        cgle_i_s = cs("cgle_i")
        sgs_s = cs("sgs")
        sms_s = cs("sms")
        WsN_s = cs("WsN").rearrange("p (o u) -> p o u", o=16)
        WsD_s = cs("WsD").rearrange("p (o u) -> p o u", o=16)
        dwp_s = cs("dwp")

        # ---- sensory precompute: NS/DS [inter, (b,t)] in PSUM ----
        zs = zp.tile([SENSORY, 16, BT], F, tag="zs")
        x_bc = xT_s[:, :].unsqueeze(1).broadcast_to([SENSORY, 16, BT])
        sgs_bc = sgs_s.unsqueeze(2).broadcast_to([SENSORY, 16, BT])
        sms_bc = sms_s.unsqueeze(2).broadcast_to([SENSORY, 16, BT])
        nc.vector.tensor_mul(zs, x_bc, sgs_bc)
        nc.vector.tensor_sub(zs, zs, sms_bc)
        sigs = sp.tile([SENSORY, 16, BT], F, tag="sigs")
        nc.scalar.activation(sigs, zs, ACTF.Sigmoid)
        NS = ps_sens.tile([INTER, BT], F, tag="NS")
        DS = ps_sens.tile([INTER, BT], F, tag="DS")
        for o in range(16):
            nc.tensor.matmul(
                NS, lhsT=WsN_s[:, o, :], rhs=sigs[:, o, :],
                start=(o == 0), stop=(o == 15),
            )
        for o in range(16):
            nc.tensor.matmul(
                DS, lhsT=WsD_s[:, o, :], rhs=sigs[:, o, :],
                start=(o == 0), stop=(o == 15),
            )

        # ---- inter trajectory precompute ----
        # per (p,b,t): den = DS + cge; a = cmt_i/den; c = (NS+glvl)/den
        # v(t,u) = a_t^u v(t,0) + (sum_{k<u} a_t^k) c_t; v(t+1,0) from u=6.
        aP = pre.tile([INTER, B_LOC, T_LEN], F, tag="aP")
        cP = pre.tile([INTER, B_LOC, T_LEN], F, tag="cP")
        den_i = pre.tile([INTER, B_LOC, T_LEN], F, tag="den_i")
        NS3 = NS.rearrange("p (b t) -> p b t", b=B_LOC)
        DS3 = DS.rearrange("p (b t) -> p b t", b=B_LOC)
        nc.vector.tensor_scalar_add(den_i, DS3, cgle_i_s)
        rD = pre.tile([INTER, B_LOC, T_LEN], F, tag="rD")
        nc.vector.reciprocal(rD, den_i)
        nc.vector.tensor_scalar_mul(aP, rD, cmt_i_s)
        nc.vector.scalar_tensor_tensor(
            cP, in0=NS3, scalar=glvl_i_s, in1=rD, op0=A.add, op1=A.mult
        )
        # powers a^u (u=0..6) and csum_u = (sum_{k<u} a^k) * c
        apw = pre.tile([INTER, B_LOC, T_LEN, ODE_UNFOLDS + 1], F, tag="apw")
        csm = pre.tile([INTER, B_LOC, T_LEN, ODE_UNFOLDS + 1], F, tag="csm")
        ssum = pre.tile([INTER, B_LOC, T_LEN], F, tag="ssum")
        nc.vector.memset(apw[:, :, :, 0], 1.0)
        nc.vector.memset(csm[:, :, :, 0], 0.0)
        nc.vector.memset(ssum, 1.0)  # S_1 = a^0
        for u in range(1, ODE_UNFOLDS + 1):
            # v(t,u) = a^u v(t,0) + S_u c,  S_u = sum_{j<u} a^j
            nc.vector.tensor_mul(apw[:, :, :, u], apw[:, :, :, u - 1], aP)
            nc.vector.tensor_mul(csm[:, :, :, u], ssum, cP)
            if u < ODE_UNFOLDS:
                nc.vector.tensor_add(ssum, ssum, apw[:, :, :, u])
        v0 = pre.tile([INTER, B_LOC, T_LEN + 1], F, tag="v0")
        nc.vector.memset(v0[:, :, 0], 0.0)
        for t in range(T_LEN):
            nc.vector.tensor_mul(
                v0[:, :, t + 1], v0[:, :, t], apw[:, :, t, ODE_UNFOLDS]
            )
            nc.vector.tensor_add(
                v0[:, :, t + 1], v0[:, :, t + 1], csm[:, :, t, ODE_UNFOLDS]
            )
        VI = pre.tile([INTER, B_LOC, T_LEN, ODE_UNFOLDS], F, tag="VI")
        v0bc = v0[:, :, 0:T_LEN].unsqueeze(3).broadcast_to(
            [INTER, B_LOC, T_LEN, ODE_UNFOLDS]
        )
        nc.vector.tensor_mul(VI, apw[:, :, :, 0:ODE_UNFOLDS], v0bc)
        nc.vector.tensor_add(VI, VI, csm[:, :, :, 0:ODE_UNFOLDS])

        zlhs = const.tile([INTER, NPOST], F, tag="zlhs")
        nc.vector.memset(zlhs, 0.0)

        # ---- state: hist [97, BH, 33] per half (cmd 0:64, motor 64:96, ones 96)
        hists = []
        for h in range(NH):
            hh = state.tile([NPOST + 1, BH, T_LEN + 1], F, tag=f"hist{h}")
            nc.vector.memset(hh, 0.0)
            nc.vector.memset(hh[NPOST : NPOST + 1, :, :], 1.0)
            hists.append(hh)

        # ---- main scan ----
        n_steps = T_LEN * ODE_UNFOLDS
        ps_prev = [None, None]

        for k in range(n_steps):
            t = k // ODE_UNFOLDS
            u = k % ODE_UNFOLDS
            # state s_j lives in col (j+5)//6 (s_0 = col 0 zeros); iteration k
            # first computes s_k from s_{k-1} + ps(k-1), then z2 reads s_k.
            rdcol = (k + 4) // 6  # col of s_{k-1}
            scol = (k + 5) // 6  # col of s_k (write target + z2 source)

            # 4b-wide inter feed, batched per t (all 6 unfolds; off-chain)
            if u == 0:
                z1 = zp.tile([INTER, ODE_UNFOLDS, S1, B_LOC], F, tag="z1")
                vi_bc = (
                    VI[:, :, t, :]
                    .rearrange("p b u -> p u b")
                    .unsqueeze(2)
                    .broadcast_to([INTER, ODE_UNFOLDS, S1, B_LOC])
                )
                sg1_bc = sg1_s.unsqueeze(1).broadcast_to(
                    [INTER, ODE_UNFOLDS, S1, B_LOC]
                )
                sm1_bc = sm1_s.unsqueeze(1).broadcast_to(
                    [INTER, ODE_UNFOLDS, S1, B_LOC]
                )
                nc.gpsimd.tensor_tensor(z1, sg1_bc, vi_bc, op=A.mult)
                nc.gpsimd.tensor_tensor(z1, z1, sm1_bc, op=A.subtract)
                sig1_t = sp.tile([INTER, ODE_UNFOLDS, S1, B_LOC], F, tag="sig1")
                nc.scalar.activation(sig1_t, z1, ACTF.Sigmoid)
            sig1 = sig1_t[:, u]

            # state update s_k = ps_n * (1/ps_d); two ops because a DVE
            # instruction may read only one PSUM operand. The reciprocal
            # depends only on the den side (finishes first), so it hides.
            if k > 0:
                rds = []
                for h in range(NH):
                    rd = up.tile([NPOST, BH], F, tag=f"rd{h}")
                    nc.vector.reciprocal(rd, ps_prev[h][:, 1, :])
                    rds.append(rd)
                for h in range(NH):
                    nc.vector.tensor_tensor(
                        hists[h][0:NPOST, :, scol], ps_prev[h][:, 0, :],
                        rds[h], op=A.mult,
                    )
            z2s_, sig2s_ = [], []
            for h in range(NH):
                z2 = zp.tile([COMMAND, S2, BH], F, tag=f"z2{h}")
                vcb = hists[h][0:COMMAND, :, scol].unsqueeze(1).broadcast_to(
                    [COMMAND, S2, BH]
                )
                nc.vector.tensor_tensor(z2, sg2_s, vcb, op=A.mult)
                z2s_.append(z2)
            for h in range(NH):
                nc.vector.tensor_tensor(z2s_[h], z2s_[h], sm2_s, op=A.subtract)
            for h in range(NH):
                sig2 = sp.tile([COMMAND, S2, BH], F, tag=f"sig2{h}")
                nc.scalar.activation(sig2, z2s_[h], ACTF.Sigmoid)
                sig2s_.append(sig2)

            for h in range(NH):
                hh = hists[h]
                bsl = slice(h * BH, (h + 1) * BH)
                sig2 = sig2s_[h]
                ps2 = ps_work.tile([NPOST, 2, BH], F, tag=f"ps{h}")
                # one full-tile zeroing matmul opens the group (start=True on
                # partial regions would be two opens); then pure accumulation.
                nc.tensor.matmul(
                    ps2, lhsT=zlhs, rhs=sg1_s[:, 0:2, 0:BH],
                    start=True, stop=False, skip_group_check=True,
                )
                # den side first (ready earlier), num side last gates the div
                for s in range(S1):
                    nc.tensor.matmul(
                        ps2[:, 1, :], lhsT=W1d_s[:, s, :], rhs=sig1[:, s, bsl],
                        start=False, stop=False, skip_group_check=True,
                    )
                for s in range(S1):
                    nc.tensor.matmul(
                        ps2[:, 0, :], lhsT=W1n_s[:, s, :], rhs=sig1[:, s, bsl],
                        start=False, stop=False, skip_group_check=True,
                    )
                nc.tensor.matmul(
                    ps2[:, 0, :], lhsT=CMTD_s, rhs=hh[0:NPOST, :, scol],
                    start=False, stop=False, skip_group_check=True,
                )
                for s in range(S2):
                    nc.tensor.matmul(
                        ps2[:, 1, :], lhsT=W2d_s[:, s, :], rhs=sig2[:, s, :],
                        start=False, stop=False, skip_group_check=True,
                    )
                for s in range(S2):
                    nc.tensor.matmul(
                        ps2[:, 0, :], lhsT=W2n_s[:, s, :], rhs=sig2[:, s, :],
                        start=False, stop=(s == S2 - 1), skip_group_check=True,
                    )
                ps_prev[h] = ps2

        # final state update (k = n_steps): s_192 from s_191
        k = n_steps
        for h in range(NH):
            rd = up.tile([NPOST, BH], F, tag=f"rd{h}")
            nc.vector.reciprocal(rd, ps_prev[h][:, 1, :])
            nc.vector.tensor_tensor(
                hists[h][0:NPOST, :, (k + 5) // 6], ps_prev[h][:, 0, :],
                rd, op=A.mult,
            )

        # ---- output: per half out[(bh,t), o] = hist[64:97].T @ dwp ----
        od2 = out_d.rearrange("b t o -> (b t) o")
        for i in range(2):
            ps_o = ps_out.tile([2 * T_LEN, OUT_LEN], F, tag=f"po{i}")
            for j in range(2):
                bi = 2 * i + j  # global batch index
                h, b = bi // BH, bi % BH
                # rows 64:97 = motor (64:96) + ones (96); cols 1:33
                lh = hists[h][COMMAND:, b, 1 : T_LEN + 1]
                nc.tensor.matmul(
                    ps_o[j * T_LEN : (j + 1) * T_LEN, :], lhsT=lh, rhs=dwp_s,
                    start=True, stop=True, skip_group_check=True,
                )
            sb_o = const.tile([2 * T_LEN, OUT_LEN], F, tag=f"sbo{i}")
            nc.scalar.copy(sb_o, ps_o)
            nc.sync.dma_start(
                out=od2[2 * i * T_LEN : (2 * i + 2) * T_LEN, :], in_=sb_o
            )

    orig_json = nc.to_json_bytes
    nc.to_json_bytes = lambda: _hoist_embedded_waits(orig_json())
    return nc


def _prep_tables(inp):
    """Host-side parameter/layout prep (pure transposes/products of params)."""
    g = {k: np.asarray(v, np.float32) for k, v in inp.items()}
    M, C, I = MOTOR, COMMAND, INTER
    sl_m = slice(0, M)
    sl_c = slice(M, M + C)
    sl_i = slice(M + C, UNITS)

    sigma, mu, w = g["sigma"], g["mu"], g["w"]
    erev, mask = g["erev"], g["mask"]
    cmt = g["cm"] * float(ODE_UNFOLDS)
    gl, vl = g["gleak"], g["vleak"]
    glvl = gl * vl
    cge = cmt + gl + EPS

    # post-row mapping: cmd unit j (abs M..M+C) -> row j-M; motor j -> 64+j
    def post_row(j):
        return j - M if j >= M else C + j

    glvl_cm = np.zeros(NPOST, np.float32)
    cge_cm = np.zeros(NPOST, np.float32)
    cmt_cm = np.zeros(NPOST, np.float32)
    for j in range(M + C):
        q = post_row(j)
        glvl_cm[q] = glvl[j]
        cge_cm[q] = cge[j]
        cmt_cm[q] = cmt[j]

    tb = {}
    # inter slots (pre rows sl_i; posts all cmd)
    sg1 = np.zeros((I, S1), np.float32)
    sm1 = np.zeros((I, S1), np.float32)
    W1n = np.zeros((I, S1, NPOST), np.float32)
    W1d = np.zeros((I, S1, NPOST), np.float32)
    for p in range(I):
        pre = M + C + p
        tgt = np.nonzero(mask[pre])[0]
        assert len(tgt) == S1 and tgt.min() >= M and tgt.max() < M + C
        for s, j in enumerate(tgt):
            sg1[p, s] = sigma[pre, j]
            sm1[p, s] = sigma[pre, j] * mu[pre, j]
            W1n[p, s, post_row(j)] = w[pre, j] * erev[pre, j]
            W1d[p, s, post_row(j)] = w[pre, j]
    tb["sg1"] = np.repeat(sg1[:, :, None], B_LOC, 2).reshape(I, -1)
    tb["sm1"] = np.repeat(sm1[:, :, None], B_LOC, 2).reshape(I, -1)
    tb["W1n"] = W1n.reshape(I, -1)
    tb["W1d"] = W1d.reshape(I, -1)

    # cmd slots (pre rows sl_c; posts cmd+motor); slot S2-1 may carry bias
    deg = np.array([np.count_nonzero(mask[M + c, : M + C]) for c in range(C)])
    assert deg.max() <= S2
    cmin = int(np.argmin(deg))
    assert deg[cmin] < S2
    sg2 = np.zeros((C, S2), np.float32)
    sm2 = np.zeros((C, S2), np.float32)
    W2n = np.zeros((C, S2, NPOST), np.float32)
    W2d = np.zeros((C, S2, NPOST), np.float32)
    for c in range(C):
        pre = M + c
        tgt = np.nonzero(mask[pre, : M + C])[0]
        for s, j in enumerate(tgt):
            sg2[c, s] = sigma[pre, j]
            sm2[c, s] = sigma[pre, j] * mu[pre, j]
            q = post_row(j)
            W2n[c, s, q] = w[pre, j] * erev[pre, j]
            W2d[c, s, q] = w[pre, j]
    # bias pseudo-synapse: constant sigmoid 1 on (cmin, S2-1)
    assert np.count_nonzero(W2d[cmin, S2 - 1]) == 0
    sg2[cmin, S2 - 1] = 0.0
    sm2[cmin, S2 - 1] = -40.0  # z = -sm -> sigmoid(40) = 1
    W2n[cmin, S2 - 1, :] = glvl_cm
    W2d[cmin, S2 - 1, :] = cge_cm
    tb["sg2"] = np.repeat(sg2[:, :, None], BH, 2).reshape(C, -1)
    tb["sm2"] = np.repeat(sm2[:, :, None], BH, 2).reshape(C, -1)
    tb["W2n"] = W2n.reshape(C, -1)
    tb["W2d"] = W2d.reshape(C, -1)

    tb["CMTD"] = np.diag(cmt_cm).astype(np.float32)
    tb["cmt_cm"] = cmt_cm[:, None]
    tb["cmt_i"] = cmt[sl_i][:, None]
    tb["glvl_i"] = glvl[sl_i][:, None]
    tb["cgle_i"] = cge[sl_i][:, None]

    # sensory fan-out (16 targets per sensory unit, all inter)
    smask, serev = g["sensory_mask"], g["sensory_erev"]
    ssig, smu_s, sw = g["sensory_sigma"], g["sensory_mu"], g["sensory_w"]
    iw, ib = g["input_w"], g["input_b"]
    sgs = np.zeros((SENSORY, 16), np.float32)
    sms = np.zeros((SENSORY, 16), np.float32)
    WsN = np.zeros((SENSORY, 16, I), np.float32)
    WsD = np.zeros((SENSORY, 16, I), np.float32)
    for s in range(SENSORY):
        tgt = np.nonzero(smask[s])[0]
        assert len(tgt) == 16 and tgt.min() >= M + C
        for o, uu in enumerate(tgt):
            ul = uu - (M + C)
            sgs[s, o] = ssig[s, uu] * iw[s]
            sms[s, o] = ssig[s, uu] * (smu_s[s, uu] - ib[s])
            WsN[s, o, ul] = sw[s, uu] * serev[s, uu]
            WsD[s, o, ul] = sw[s, uu] * smask[s, uu]
    tb["sgs"], tb["sms"] = sgs, sms
    tb["WsN"] = WsN.reshape(SENSORY, -1)
    tb["WsD"] = WsD.reshape(SENSORY, -1)

    # output: motor rows of hist are post_row order 64..95 = motor unit j
    ow, ob = g["output_w"], g["output_b"]
    dw, db = g["dense_w"], g["dense_b"]
    dwp = np.zeros((M + 1, OUT_LEN), np.float32)
    dwp[:M] = ow[:, None] * dw
    dwp[M] = db + ob @ dw
    tb["dwp"] = dwp

    CTa = np.zeros((128, _K), np.float32)
    for n, r, c in _SPEC_ORDER:
        _, c0, cn = _SPEC_OFF[n]
        a = tb[n]
        assert a.shape == (r, cn), (n, a.shape, (r, cn))
        r0 = COMMAND if n == "dwp" else 0  # dwp rides rows 64:97
        CTa[r0 : r0 + r, c0 : c0 + cn] = a
    return CTa, g


def kernel(**inputs):
    from concourse.bass_utils import run_bass_kernel_spmd

    if "nc" not in _CACHE:
        _CACHE["nc"] = _build_program()
    nc = _CACHE["nc"]

    CTa, g = _prep_tables(inputs)
    x = g["inputs"]  # [B, T, S]
    in_maps = []
    for c in range(N_CORES):
        xc = x[c * B_LOC : (c + 1) * B_LOC]  # [4, T, S]
        xTc = np.ascontiguousarray(
            np.transpose(xc, (2, 0, 1)).reshape(SENSORY, BT)
        )
        in_maps.append({"xT": xTc, "CT": CTa})

    res = run_bass_kernel_spmd(nc, in_maps, list(range(N_CORES)))
    out = np.concatenate([res.results[c]["out"] for c in range(N_CORES)], axis=0)
    return out.astype(np.float32)


if __name__ == "__main__":
    import reference

    inp = {k: np.asarray(v) for k, v in reference.setup_inputs().items()}
    got = kernel(**inp)
    want = np.asarray(reference.reference(**reference.setup_inputs()))
    err = np.abs(got - want).max() / (np.abs(want).max() + 1e-12)
    print("Relative error:", err)


# revision 22
# speedup vs baseline: 2.1449x; 1.0000x over previous
"""Trainium2 Bass kernel for a Neural Circuit Policies (LTC) cell.

Strategy (v2): data-parallel over batch (32 -> 8 cores x 4). Per core the
T=32 x 6-unfold scan runs fully unrolled with a 2-way batch-pair interleave
to hide cross-engine latency.

Key structure:
- Inter neurons receive no recurrent synapses, so their whole 192-step
  trajectory is an affine recurrence v' = A_t v + C_t with per-step (t)
  coefficients from the sensory tables. It is precomputed OUTSIDE the scan
  (closed-form powers of A_t expand the 6 unfolds per step), removing inter
  from the serial chain.
- Recurrent reductions use per-synapse-slot scatter matmuls (PE issue is
  ~3.5ns/matmul): inter->cmd uses 16 slots with the erev sign folded into a
  (num,den) rhs pair (premultiplied on gpsimd); cmd->cmd/motor uses 17 slots
  with separate num/den matmuls (sign lives in the lhsT). PSUM receives
  num/den [96, b] directly -- no diagonal extraction.
- Leak/eps biases ride a constant-sigmoid slot (z=40 -> sig=1) of a
  low-degree cmd unit.
- cmd+motor state lives in a [97, b, 33] history tile (ones row for the
  output bias); each unfold's divide writes column ceil(k/6), so the motor
  trajectory needed by the output matmul materializes with zero extra ops.
- Output: per half one matmul (hist motor rows + ones row as lhsT) x packed
  dense weights.
"""

import numpy as np

MOTOR, COMMAND, INTER = 32, 64, 128
UNITS = MOTOR + COMMAND + INTER  # order: motor, command, inter
SENSORY = 64
ODE_UNFOLDS = 6
EPS = 1e-8
B_FULL, T_LEN, OUT_LEN = 32, 32, 32
N_CORES = 8
B_LOC = B_FULL // N_CORES  # 4
BT = B_LOC * T_LEN  # 128
NH = 2  # interleaved halves
BH = B_LOC // NH  # batch per half
NPOST = COMMAND + MOTOR  # 96
S1 = 16  # inter out-slots (exact fanout)
S2 = 17  # cmd out-slots (max out-degree, slot also carries the bias)

# name -> (rows, free_elems); all f32, packed into one [128, K] DMA
_SPEC_ORDER = [
    ("sg1", INTER, S1 * B_LOC),        # [p, s, b] replicated over b
    ("sm1", INTER, S1 * B_LOC),
    ("W1n", INTER, S1 * NPOST),        # signed inter lhsT (w*erev)
    ("W1d", INTER, S1 * NPOST),        # unsigned inter lhsT (w*mask)
    ("sg2", COMMAND, S2 * BH),         # [c, s, bh] replicated over bh
    ("sm2", COMMAND, S2 * BH),
    ("W2n", COMMAND, S2 * NPOST),      # signed lhsT (w*erev [+ bias row])
    ("W2d", COMMAND, S2 * NPOST),      # unsigned lhsT (w*mask [+ bias row])
    ("CMTD", NPOST, NPOST),            # diag(cmt) lhsT folding cmt*v into ps
    ("cmt_cm", NPOST, 1),
    ("cmt_i", INTER, 1),
    ("glvl_i", INTER, 1),
    ("cgle_i", INTER, 1),
    ("sgs", SENSORY, 16),
    ("sms", SENSORY, 16),
    ("WsN", SENSORY, 16 * INTER),
    ("WsD", SENSORY, 16 * INTER),
    ("dwp", MOTOR + 1, OUT_LEN),
]
_SPEC_OFF = {}
_K = 0
for _n, _r, _c in _SPEC_ORDER:
    _SPEC_OFF[_n] = (_r, _K, _c)
    _K += _c

_CACHE = {}


def _hoist_embedded_waits(bir_bytes):
    """This walrus build rejects instructions with multiple embedded sync
    waits; hoist every embedded wait into a standalone EventSemaphore
    instruction placed just before it on the same engine stream."""
    import json as _json

    ctr = [0]

    def fix_block(bb):
        out = []
        for ins in bb.get("instructions", []):
            si = ins.get("sync_info")
            if si and si.get("on_wait"):
                for w in si["on_wait"]:
                    ctr[0] += 1
                    out.append({
                        "debug": ins.get("debug", 0),
                        "engine": ins["engine"],
                        "ins": [],
                        "outs": [],
                        "name": f"EVW-{ctr[0]}",
                        "opcode": "EventSemaphore",
                        "sync_info": {"on_update": [], "on_wait": [w]},
                    })
                si["on_wait"] = []
            out.append(ins)
        bb["instructions"] = out
        for sub in bb.get("blocks", []) or []:
            fix_block(sub)

    m = _json.loads(bir_bytes)
    for fn in m["functions"]:
        for bb in fn.get("blocks", []):
            fix_block(bb)
    return _json.dumps(m).encode()


def _build_program():
    from contextlib import ExitStack

    import concourse.bass as bass
    import concourse.tile as tile
    import concourse.mybir as mybir

    F = mybir.dt.float32
    A = mybir.AluOpType
    ACTF = mybir.ActivationFunctionType

    nc = bass.Bass("TRN2", target_bir_lowering=False, debug=False)

    xT = nc.dram_tensor("xT", [SENSORY, BT], F, kind="ExternalInput").ap()
    CT = nc.dram_tensor("CT", [128, _K], F, kind="ExternalInput").ap()
    out_d = nc.dram_tensor(
        "out", [B_LOC, T_LEN, OUT_LEN], F, kind="ExternalOutput"
    ).ap()

    with tile.TileContext(nc) as tc, ExitStack() as ctx:
        const = ctx.enter_context(tc.tile_pool(name="const", bufs=1))
        state = ctx.enter_context(tc.tile_pool(name="state", bufs=1))
        pre = ctx.enter_context(tc.tile_pool(name="pre", bufs=2))
        zp = ctx.enter_context(tc.tile_pool(name="zp", bufs=3))
        sp = ctx.enter_context(tc.tile_pool(name="sp", bufs=3))
        up = ctx.enter_context(tc.tile_pool(name="up", bufs=3))
        ps_sens = ctx.enter_context(tc.tile_pool(name="ps_sens", bufs=1, space="PSUM"))
        ps_work = ctx.enter_context(tc.tile_pool(name="ps_work", bufs=2, space="PSUM"))
        ps_out = ctx.enter_context(tc.tile_pool(name="ps_out", bufs=1, space="PSUM"))

        ct = const.tile([128, _K], F, tag="ct")
        # split the big constant DMA across queues for transfer parallelism
        nch = (_K + 3) // 4
        for ci in range(4):
            c0, c1 = ci * nch, min((ci + 1) * nch, _K)
            nc.gpsimd.dma_start(out=ct[:, c0:c1], in_=CT[:, c0:c1])
        xT_s = const.tile([SENSORY, BT], F, tag="xT")
        nc.gpsimd.dma_start(out=xT_s, in_=xT)
        tc.strict_bb_all_engine_barrier()

        def cs(name):
            r, c0, cn = _SPEC_OFF[name]
            if name == "dwp":  # aligned with hist motor rows for the out matmul
                return ct[COMMAND : COMMAND + r, c0 : c0 + cn]
            return ct[0:r, c0 : c0 + cn]

        sg1_s = cs("sg1").rearrange("p (s b) -> p s b", s=S1)
        sm1_s = cs("sm1").rearrange("p (s b) -> p s b", s=S1)
        W1n_s = cs("W1n").rearrange("p (s q) -> p s q", s=S1)
        W1d_s = cs("W1d").rearrange("p (s q) -> p s q", s=S1)
        sg2_s = cs("sg2").rearrange("p (s b) -> p s b", s=S2)
        sm2_s = cs("sm2").rearrange("p (s b) -> p s b", s=S2)
        W2n_s = cs("W2n").rearrange("p (s q) -> p s q", s=S2)
        W2d_s = cs("W2d").rearrange("p (s q) -> p s q", s=S2)
        CMTD_s = cs("CMTD")
        cmt_cm_s = cs("cmt_cm")
        cmt_i_s = cs("cmt_i")
        glvl_i_s = cs("glvl_i")
        cgle_i_s = cs("cgle_i")
        sgs_s = cs("sgs")
        sms_s = cs("sms")
        WsN_s = cs("WsN").rearrange("p (o u) -> p o u", o=16)
        WsD_s = cs("WsD").rearrange("p (o u) -> p o u", o=16)
        dwp_s = cs("dwp")

        # ---- sensory precompute: NS/DS [inter, (b,t)] in PSUM ----
        zs = zp.tile([SENSORY, 16, BT], F, tag="zs")
        x_bc = xT_s[:, :].unsqueeze(1).broadcast_to([SENSORY, 16, BT])
        sgs_bc = sgs_s.unsqueeze(2).broadcast_to([SENSORY, 16, BT])
        sms_bc = sms_s.unsqueeze(2).broadcast_to([SENSORY, 16, BT])
        nc.vector.tensor_mul(zs, x_bc, sgs_bc)
        nc.vector.tensor_sub(zs, zs, sms_bc)
        sigs = sp.tile([SENSORY, 16, BT], F, tag="sigs")
        nc.scalar.activation(sigs, zs, ACTF.Sigmoid)
        NS = ps_sens.tile([INTER, BT], F, tag="NS")
        DS = ps_sens.tile([INTER, BT], F, tag="DS")
        for o in range(16):
            nc.tensor.matmul(
                NS, lhsT=WsN_s[:, o, :], rhs=sigs[:, o, :],
                start=(o == 0), stop=(o == 15),
            )
        for o in range(16):
            nc.tensor.matmul(
                DS, lhsT=WsD_s[:, o, :], rhs=sigs[:, o, :],
                start=(o == 0), stop=(o == 15),
            )

        # ---- inter trajectory precompute ----
        # per (p,b,t): den = DS + cge; a = cmt_i/den; c = (NS+glvl)/den
        # v(t,u) = a_t^u v(t,0) + (sum_{k<u} a_t^k) c_t; v(t+1,0) from u=6.
        aP = pre.tile([INTER, B_LOC, T_LEN], F, tag="aP")
        cP = pre.tile([INTER, B_LOC, T_LEN], F, tag="cP")
        den_i = pre.tile([INTER, B_LOC, T_LEN], F, tag="den_i")
        NS3 = NS.rearrange("p (b t) -> p b t", b=B_LOC)
        DS3 = DS.rearrange("p (b t) -> p b t", b=B_LOC)
        nc.vector.tensor_scalar_add(den_i, DS3, cgle_i_s)
        rD = pre.tile([INTER, B_LOC, T_LEN], F, tag="rD")
        nc.vector.reciprocal(rD, den_i)
        nc.vector.tensor_scalar_mul(aP, rD, cmt_i_s)
        nc.vector.scalar_tensor_tensor(
            cP, in0=NS3, scalar=glvl_i_s, in1=rD, op0=A.add, op1=A.mult
        )
        # powers a^u (u=0..6) and csum_u = (sum_{k<u} a^k) * c
        apw = pre.tile([INTER, B_LOC, T_LEN, ODE_UNFOLDS + 1], F, tag="apw")
        csm = pre.tile([INTER, B_LOC, T_LEN, ODE_UNFOLDS + 1], F, tag="csm")
        ssum = pre.tile([INTER, B_LOC, T_LEN], F, tag="ssum")
        nc.vector.memset(apw[:, :, :, 0], 1.0)
        nc.vector.memset(csm[:, :, :, 0], 0.0)
        nc.vector.memset(ssum, 1.0)  # S_1 = a^0
        for u in range(1, ODE_UNFOLDS + 1):
            # v(t,u) = a^u v(t,0) + S_u c,  S_u = sum_{j<u} a^j
            nc.vector.tensor_mul(apw[:, :, :, u], apw[:, :, :, u - 1], aP)
            nc.vector.tensor_mul(csm[:, :, :, u], ssum, cP)
            if u < ODE_UNFOLDS:
                nc.vector.tensor_add(ssum, ssum, apw[:, :, :, u])
        v0 = pre.tile([INTER, B_LOC, T_LEN + 1], F, tag="v0")
        nc.vector.memset(v0[:, :, 0], 0.0)
        for t in range(T_LEN):
            nc.vector.tensor_mul(
                v0[:, :, t + 1], v0[:, :, t], apw[:, :, t, ODE_UNFOLDS]
            )
            nc.vector.tensor_add(
                v0[:, :, t + 1], v0[:, :, t + 1], csm[:, :, t, ODE_UNFOLDS]
            )
        VI = pre.tile([INTER, B_LOC, T_LEN, ODE_UNFOLDS], F, tag="VI")
        v0bc = v0[:, :, 0:T_LEN].unsqueeze(3).broadcast_to(
            [INTER, B_LOC, T_LEN, ODE_UNFOLDS]
        )
        nc.vector.tensor_mul(VI, apw[:, :, :, 0:ODE_UNFOLDS], v0bc)
        nc.vector.tensor_add(VI, VI, csm[:, :, :, 0:ODE_UNFOLDS])

        zlhs = const.tile([INTER, NPOST], F, tag="zlhs")
        nc.vector.memset(zlhs, 0.0)

        # ---- state: hist [97, BH, 33] per half (cmd 0:64, motor 64:96, ones 96)
        hists = []
        for h in range(NH):
            hh = state.tile([NPOST + 1, BH, T_LEN + 1], F, tag=f"hist{h}")
            nc.vector.memset(hh, 0.0)
            nc.vector.memset(hh[NPOST : NPOST + 1, :, :], 1.0)
            hists.append(hh)

        # ---- main scan ----
        n_steps = T_LEN * ODE_UNFOLDS
        ps_prev = [None, None]

        for k in range(n_steps):
            t = k // ODE_UNFOLDS
            u = k % ODE_UNFOLDS
            # state s_j lives in col (j+5)//6 (s_0 = col 0 zeros); iteration k
            # first computes s_k from s_{k-1} + ps(k-1), then z2 reads s_k.
            rdcol = (k + 4) // 6  # col of s_{k-1}
            scol = (k + 5) // 6  # col of s_k (write target + z2 source)

            # 4b-wide inter feed, batched per t (all 6 unfolds; off-chain)
            if u == 0:
                z1 = zp.tile([INTER, ODE_UNFOLDS, S1, B_LOC], F, tag="z1")
                vi_bc = (
                    VI[:, :, t, :]
                    .rearrange("p b u -> p u b")
                    .unsqueeze(2)
                    .broadcast_to([INTER, ODE_UNFOLDS, S1, B_LOC])
                )
                sg1_bc = sg1_s.unsqueeze(1).broadcast_to(
                    [INTER, ODE_UNFOLDS, S1, B_LOC]
                )
                sm1_bc = sm1_s.unsqueeze(1).broadcast_to(
                    [INTER, ODE_UNFOLDS, S1, B_LOC]
                )
                nc.gpsimd.tensor_tensor(z1, sg1_bc, vi_bc, op=A.mult)
                nc.gpsimd.tensor_tensor(z1, z1, sm1_bc, op=A.subtract)
                sig1_t = sp.tile([INTER, ODE_UNFOLDS, S1, B_LOC], F, tag="sig1")
                nc.scalar.activation(sig1_t, z1, ACTF.Sigmoid)
            sig1 = sig1_t[:, u]

            # state update s_k = ps_n * (1/ps_d); two ops because a DVE
            # instruction may read only one PSUM operand. The reciprocal
            # depends only on the den side (finishes first), so it hides.
            if k > 0:
                rds = []
                for h in range(NH):
                    rd = up.tile([NPOST, BH], F, tag=f"rd{h}")
                    nc.vector.reciprocal(rd, ps_prev[h][:, 1, :])
                    rds.append(rd)
                for h in range(NH):
                    nc.vector.tensor_tensor(
                        hists[h][0:NPOST, :, scol], ps_prev[h][:, 0, :],
                        rds[h], op=A.mult,
                    )
            z2s_, sig2s_ = [], []
            for h in range(NH):
                z2 = zp.tile([COMMAND, S2, BH], F, tag=f"z2{h}")
                vcb = hists[h][0:COMMAND, :, scol].unsqueeze(1).broadcast_to(
                    [COMMAND, S2, BH]
                )
                nc.vector.tensor_tensor(z2, sg2_s, vcb, op=A.mult)
                z2s_.append(z2)
            for h in range(NH):
                nc.vector.tensor_tensor(z2s_[h], z2s_[h], sm2_s, op=A.subtract)
            for h in range(NH):
                sig2 = sp.tile([COMMAND, S2, BH], F, tag=f"sig2{h}")
                nc.scalar.activation(sig2, z2s_[h], ACTF.Sigmoid)
                sig2s_.append(sig2)

            for h in range(NH):
                hh = hists[h]
                bsl = slice(h * BH, (h + 1) * BH)
                sig2 = sig2s_[h]
                ps2 = ps_work.tile([NPOST, 2, BH], F, tag=f"ps{h}")
                # one full-tile zeroing matmul opens the group (start=True on
                # partial regions would be two opens); then pure accumulation.
                nc.tensor.matmul(
                    ps2, lhsT=zlhs, rhs=sg1_s[:, 0:2, 0:BH],
                    start=True, stop=False, skip_group_check=True,
                )
                # den side first (ready earlier), num side last gates the div
                for s in range(S1):
                    nc.tensor.matmul(
                        ps2[:, 1, :], lhsT=W1d_s[:, s, :], rhs=sig1[:, s, bsl],
                        start=False, stop=False, skip_group_check=True,
                    )
                for s in range(S1):
                    nc.tensor.matmul(
                        ps2[:, 0, :], lhsT=W1n_s[:, s, :], rhs=sig1[:, s, bsl],
                        start=False, stop=False, skip_group_check=True,
                    )
                nc.tensor.matmul(
                    ps2[:, 0, :], lhsT=CMTD_s, rhs=hh[0:NPOST, :, scol],
                    start=False, stop=False, skip_group_check=True,
                )
                for s in range(S2):
                    nc.tensor.matmul(
                        ps2[:, 1, :], lhsT=W2d_s[:, s, :], rhs=sig2[:, s, :],
                        start=False, stop=False, skip_group_check=True,
                    )
                for s in range(S2):
                    nc.tensor.matmul(
                        ps2[:, 0, :], lhsT=W2n_s[:, s, :], rhs=sig2[:, s, :],
                        start=False, stop=(s == S2 - 1), skip_group_check=True,
                    )
                ps_prev[h] = ps2

        # final state update (k = n_steps): s_192 from s_191
        k = n_steps
        for h in range(NH):
            rd = up.tile([NPOST, BH], F, tag=f"rd{h}")
            nc.vector.reciprocal(rd, ps_prev[h][:, 1, :])
            nc.vector.tensor_tensor(
                hists[h][0:NPOST, :, (k + 5) // 6], ps_prev[h][:, 0, :],
                rd, op=A.mult,
            )

        # ---- output: per half out[(bh,t), o] = hist[64:97].T @ dwp ----
        od2 = out_d.rearrange("b t o -> (b t) o")
        for i in range(2):
            ps_o = ps_out.tile([2 * T_LEN, OUT_LEN], F, tag=f"po{i}")
            for j in range(2):
                bi = 2 * i + j  # global batch index
                h, b = bi // BH, bi % BH
                # rows 64:97 = motor (64:96) + ones (96); cols 1:33
                lh = hists[h][COMMAND:, b, 1 : T_LEN + 1]
                nc.tensor.matmul(
                    ps_o[j * T_LEN : (j + 1) * T_LEN, :], lhsT=lh, rhs=dwp_s,
                    start=True, stop=True, skip_group_check=True,
                )
            sb_o = const.tile([2 * T_LEN, OUT_LEN], F, tag=f"sbo{i}")
            nc.scalar.copy(sb_o, ps_o)
            nc.sync.dma_start(
                out=od2[2 * i * T_LEN : (2 * i + 2) * T_LEN, :], in_=sb_o
            )

    orig_json = nc.to_json_bytes
    nc.to_json_bytes = lambda: _hoist_embedded_waits(orig_json())
    return nc


def _prep_tables(inp):
    """Host-side parameter/layout prep (pure transposes/products of params)."""
    g = {k: np.asarray(v, np.float32) for k, v in inp.items()}
    M, C, I = MOTOR, COMMAND, INTER
    sl_m = slice(0, M)
    sl_c = slice(M, M + C)
    sl_i = slice(M + C, UNITS)

    sigma, mu, w = g["sigma"], g["mu"], g["w"]
    erev, mask = g["erev"], g["mask"]
    cmt = g["cm"] * float(ODE_UNFOLDS)
    gl, vl = g["gleak"], g["vleak"]
    glvl = gl * vl
    cge = cmt + gl + EPS

    # post-row mapping: cmd unit j (abs M..M+C) -> row j-M; motor j -> 64+j
    def post_row(j):
        return j - M if j >= M else C + j

    glvl_cm = np.zeros(NPOST, np.float32)
    cge_cm = np.zeros(NPOST, np.float32)
    cmt_cm = np.zeros(NPOST, np.float32)
    for j in range(M + C):
        q = post_row(j)
        glvl_cm[q] = glvl[j]
        cge_cm[q] = cge[j]
        cmt_cm[q] = cmt[j]

    tb = {}
    # inter slots (pre rows sl_i; posts all cmd)
    sg1 = np.zeros((I, S1), np.float32)
    sm1 = np.zeros((I, S1), np.float32)
    W1n = np.zeros((I, S1, NPOST), np.float32)
    W1d = np.zeros((I, S1, NPOST), np.float32)
    for p in range(I):
        pre = M + C + p
        tgt = np.nonzero(mask[pre])[0]
        assert len(tgt) == S1 and tgt.min() >= M and tgt.max() < M + C
        for s, j in enumerate(tgt):
            sg1[p, s] = sigma[pre, j]
            sm1[p, s] = sigma[pre, j] * mu[pre, j]
            W1n[p, s, post_row(j)] = w[pre, j] * erev[pre, j]
            W1d[p, s, post_row(j)] = w[pre, j]
    tb["sg1"] = np.repeat(sg1[:, :, None], B_LOC, 2).reshape(I, -1)
    tb["sm1"] = np.repeat(sm1[:, :, None], B_LOC, 2).reshape(I, -1)
    tb["W1n"] = W1n.reshape(I, -1)
    tb["W1d"] = W1d.reshape(I, -1)

    # cmd slots (pre rows sl_c; posts cmd+motor); slot S2-1 may carry bias
    deg = np.array([np.count_nonzero(mask[M + c, : M + C]) for c in range(C)])
    assert deg.max() <= S2
    cmin = int(np.argmin(deg))
    assert deg[cmin] < S2
    sg2 = np.zeros((C, S2), np.float32)
    sm2 = np.zeros((C, S2), np.float32)
    W2n = np.zeros((C, S2, NPOST), np.float32)
    W2d = np.zeros((C, S2, NPOST), np.float32)
    for c in range(C):
        pre = M + c
        tgt = np.nonzero(mask[pre, : M + C])[0]
        for s, j in enumerate(tgt):
            sg2[c, s] = sigma[pre, j]
            sm2[c, s] = sigma[pre, j] * mu[pre, j]
            q = post_row(j)
            W2n[c, s, q] = w[pre, j] * erev[pre, j]
            W2d[c, s, q] = w[pre, j]
    # bias pseudo-synapse: constant sigmoid 1 on (cmin, S2-1)
    assert np.count_nonzero(W2d[cmin, S2 - 1]) == 0
    sg2[cmin, S2 - 1] = 0.0
    sm2[cmin, S2 - 1] = -40.0  # z = -sm -> sigmoid(40) = 1
    W2n[cmin, S2 - 1, :] = glvl_cm
    W2d[cmin, S2 - 1, :] = cge_cm
    tb["sg2"] = np.repeat(sg2[:, :, None], BH, 2).reshape(C, -1)
    tb["sm2"] = np.repeat(sm2[:, :, None], BH, 2).reshape(C, -1)
    tb["W2n"] = W2n.reshape(C, -1)
    tb["W2d"] = W2d.reshape(C, -1)

    tb["CMTD"] = np.diag(cmt_cm).astype(np.float32)
    tb["cmt_cm"] = cmt_cm[:, None]
    tb["cmt_i"] = cmt[sl_i][:, None]
    tb["glvl_i"] = glvl[sl_i][:, None]
    tb["cgle_i"] = cge[sl_i][:, None]

    # sensory fan-out (16 targets per sensory unit, all inter)
    smask, serev = g["sensory_mask"], g["sensory_erev"]
    ssig, smu_s, sw = g["sensory_sigma"], g["sensory_mu"], g["sensory_w"]
    iw, ib = g["input_w"], g["input_b"]
    sgs = np.zeros((SENSORY, 16), np.float32)
    sms = np.zeros((SENSORY, 16), np.float32)
    WsN = np.zeros((SENSORY, 16, I), np.float32)
    WsD = np.zeros((SENSORY, 16, I), np.float32)
    for s in range(SENSORY):
        tgt = np.nonzero(smask[s])[0]
        assert len(tgt) == 16 and tgt.min() >= M + C
        for o, uu in enumerate(tgt):
            ul = uu - (M + C)
            sgs[s, o] = ssig[s, uu] * iw[s]
            sms[s, o] = ssig[s, uu] * (smu_s[s, uu] - ib[s])
            WsN[s, o, ul] = sw[s, uu] * serev[s, uu]
            WsD[s, o, ul] = sw[s, uu] * smask[s, uu]
    tb["sgs"], tb["sms"] = sgs, sms
    tb["WsN"] = WsN.reshape(SENSORY, -1)
    tb["WsD"] = WsD.reshape(SENSORY, -1)

    # output: motor rows of hist are post_row order 64..95 = motor unit j
    ow, ob = g["output_w"], g["output_b"]
    dw, db = g["dense_w"], g["dense_b"]
    dwp = np.zeros((M + 1, OUT_LEN), np.float32)
    dwp[:M] = ow[:, None] * dw
    dwp[M] = db + ob @ dw
    tb["dwp"] = dwp

    CTa = np.zeros((128, _K), np.float32)
    for n, r, c in _SPEC_ORDER:
        _, c0, cn = _SPEC_OFF[n]
        a = tb[n]
        assert a.shape == (r, cn), (n, a.shape, (r, cn))
        r0 = COMMAND if n == "dwp" else 0  # dwp rides rows 64:97
        CTa[r0 : r0 + r, c0 : c0 + cn] = a
    return CTa, g


def kernel(**inputs):
    from concourse.bass_utils import run_bass_kernel_spmd

    if "nc" not in _CACHE:
        _CACHE["nc"] = _build_program()
    nc = _CACHE["nc"]

    CTa, g = _prep_tables(inputs)
    x = g["inputs"]  # [B, T, S]
    in_maps = []
    for c in range(N_CORES):
        xc = x[c * B_LOC : (c + 1) * B_LOC]  # [4, T, S]
        xTc = np.ascontiguousarray(
            np.transpose(xc, (2, 0, 1)).reshape(SENSORY, BT)
        )
        in_maps.append({"xT": xTc, "CT": CTa})

    res = run_bass_kernel_spmd(nc, in_maps, list(range(N_CORES)))
    out = np.concatenate([res.results[c]["out"] for c in range(N_CORES)], axis=0)
    return out.astype(np.float32)


if __name__ == "__main__":
    import reference

    inp = {k: np.asarray(v) for k, v in reference.setup_inputs().items()}
    got = kernel(**inp)
    want = np.asarray(reference.reference(**reference.setup_inputs()))
    err = np.abs(got - want).max() / (np.abs(want).max() + 1e-12)
    print("Relative error:", err)
